# revision 1
# baseline (speedup 1.0000x reference)
"""Trainium2 Bass kernel for nn_LinearTransformerLayer_44495861187342.

Reference network (see problem): SGIRA block (self-attn MHA-16h -> LN ->
cross-attn -> LN -> gate blend -> FFN(gelu) -> LN) followed by a SAIGA block
(squeeze-excite MLP -> LN -> 4-head self-attn -> LN).  With the shipped
inputs gate == 1.0, so the cross-attention branch is algebraically dead and
memory_states is unused; a general path that includes it is kept for
gate != 1.

Sharding (8 NeuronCores): core c owns 512 rows = (batch c//2, half c%2) of
the [4, 1024, 1024] input.  Every row-local op (projections, FFN, layernorm,
softmax) shards perfectly.  Self-attention K/V are recomputed per core for
the full 1024-row batch (the input is replicated host-side, so no
communication), and the mid-network activation `se` is exchanged within each
core pair via one AllGather so the SAIGA attention can see the whole batch.

Layout: activations are kept feature-major in SBUF ([feat partitions, rows
free]) so every matmul contracts the partition dim against natural-layout
weights, layernorm/softmax feature reductions become cheap PE ones-matmuls,
and the device never transposes anything (the host pre-transposes x and
post-transposes the output).  Matmuls run with operands bitcast to float32r
(full-rate on the PE, fp32 storage); everything else is fp32.
"""

import contextlib

import numpy as np

import concourse.bass as bass
import concourse.mybir as mybir
import concourse.tile as tile
from concourse import bacc
from concourse import bass_utils

F32 = mybir.dt.float32
F32R = mybir.dt.float32r
AF = mybir.ActivationFunctionType
OP = mybir.AluOpType

D = 1024          # model dim
DFF = 4096        # ffn dim
D2 = 2048         # squeeze-excite dim
S = 1024          # full sequence rows per batch
R = 512           # rows owned per core
P = 128
C = D // P        # 8 feature chunks
CFF = DFF // P    # 32
C2 = D2 // P      # 16
H_SA = 16         # SGIRA heads (hd 64)
H_SG = 4          # SAIGA heads (hd 256)
N_CORES = 8
EPS = 1e-5

_CACHE = {}


def _mm(nc, out, lhsT, rhs, start, stop):
    nc.tensor.matmul(out, lhsT.bitcast(F32R), rhs.bitcast(F32R),
                     start=start, stop=stop)


def _build(include_cross: bool, with_vbias: bool):
    nc = bacc.Bacc("TRN2", target_bir_lowering=False, debug=False,
                   num_devices=N_CORES)

    def din(name, shape):
        return nc.dram_tensor(name, shape, F32, kind="ExternalInput")

    # feature-major inputs (host pre-transposed), own 512 rows first
    xT = din("xT", [D, S])
    wq = din("wq", [D, D]); wk = din("wk", [D, D]); wv = din("wv", [D, D])
    bq = din("bq", [P, C]); bk = din("bk", [P, C])
    wo = din("wo", [D, D]); bo = din("bo", [P, C])
    w1 = din("w1", [D, DFF]); b1 = din("b1", [P, CFF])
    w2 = din("w2", [DFF, D]); b2 = din("b2", [P, C])
    exw = din("exw", [D, D2]); exb = din("exb", [P, C2])
    sqw = din("sqw", [D2, D]); sqb = din("sqb", [P, C])
    qw = din("qw", [D, D]); qb = din("qb", [P, C])
    kw = din("kw", [D, D]); kb = din("kb", [P, C])
    vw = din("vw", [D, D])
    nsg = din("nsg", [P, C]); nsb = din("nsb", [P, C])
    nfg = din("nfg", [P, C]); nfb = din("nfb", [P, C])
    nrg = din("nrg", [P, C]); nrb = din("nrb", [P, C])
    if with_vbias:
        bv = din("bv", [1, D])
        vb = din("vb", [1, D])
    if include_cross:
        mT = din("mT", [D, S])
        cwq = din("cwq", [D, D]); cwk = din("cwk", [D, D]); cwv = din("cwv", [D, D])
        cbq = din("cbq", [P, C]); cbk = din("cbk", [P, C])
        cwo = din("cwo", [D, D]); cbo = din("cbo", [P, C])
        ncg = din("ncg", [P, C]); ncb = din("ncb", [P, C])
        gate_c = din("gate_c", [P, 1])      # broadcast gate
        gate_1mc = din("gate_1mc", [P, 1])  # broadcast (1 - gate)
        if with_vbias:
            cbv = din("cbv", [1, D])

    out_d = nc.dram_tensor("out", [D, R], F32, kind="ExternalOutput")

    with tile.TileContext(nc) as tc:
        with contextlib.ExitStack() as ctx, \
             nc.allow_low_precision("float32r tiles feeding the PE"):
            const = ctx.enter_context(tc.tile_pool(name="const", bufs=1))
            wpool = ctx.enter_context(tc.tile_pool(name="wpool", bufs=3))
            # single PSUM pool, exactly 8 banks across 4 tags (bufs per tag)
            psp = ctx.enter_context(tc.tile_pool(name="psp", bufs=1,
                                                 space="PSUM"))
            tmp = ctx.enter_context(tc.tile_pool(name="tmp", bufs=2))
            small = ctx.enter_context(tc.tile_pool(name="small", bufs=1))

            def ps_mm(width=R):
                return psp.tile([P, width], F32, tag="mm", bufs=2, name="psmm")

            ones_f = const.tile([P, P], F32, tag="ones_f")
            nc.vector.memset(ones_f[:], 1.0)
            ones_col = const.tile([P, 1], F32R, tag="ones_col")
            nc.scalar.copy(ones_col[:], ones_f[:, 0:1])
            ones_row = const.tile([1, P], F32R, tag="ones_row")
            nc.scalar.copy(ones_row[:], ones_f[0:1, :])
            eps_t = const.tile([1, 1], F32, tag="eps")
            nc.vector.memset(eps_t[:], EPS)

            def load_pc(dram, dt=F32):  # small per-partition tables
                t = const.tile(list(dram.shape), dt, tag=dram.name + "_sb")
                src_ap = dram.ap().bitcast(dt) if dt is F32R else dram.ap()
                nc.sync.dma_start(out=t[:], in_=src_ap)
                return t

            bq_s = load_pc(bq); bk_s = load_pc(bk)
            bo_s = load_pc(bo); b1_s = load_pc(b1); b2_s = load_pc(b2)
            exb_s = load_pc(exb); sqb_s = load_pc(sqb)
            qb_s = load_pc(qb); kb_s = load_pc(kb)
            nsg_s = load_pc(nsg); nsb_s = load_pc(nsb)
            nfg_s = load_pc(nfg); nfb_s = load_pc(nfb)
            nrg_s = load_pc(nrg); nrb_s = load_pc(nrb)
            bv_s = load_pc(bv, F32R) if with_vbias else None
            vb_s = load_pc(vb, F32R) if with_vbias else None
            if include_cross:
                cbq_s = load_pc(cbq); cbk_s = load_pc(cbk)
                cbo_s = load_pc(cbo)
                ncg_s = load_pc(ncg); ncb_s = load_pc(ncb)
                gc_s = load_pc(gate_c); g1_s = load_pc(gate_1mc)
                cbv_s = load_pc(cbv, F32R) if with_vbias else None

            def wstat_ap(w, oc, kcs):
                # [P, kcs, 128] stationary block: w[(kc p), oc*128 + m]
                return w.ap().rearrange("(k p) m -> p k m", p=P)[:, :, oc * P:(oc + 1) * P]

            def proj(out_t, out_c, in_t, in_c, w, bias_s, rows, func=AF.Identity):
                """out_t[:, oc, r] = func(sum_k w[k, oc*128+p] * in_t[k, r] + b)."""
                for oc in range(out_c):
                    wt = wpool.tile([P, in_c, P], F32R, tag="wstat",
                                    name=f"wst_{w.name}{oc}")
                    nc.sync.dma_start(out=wt[:],
                                      in_=wstat_ap(w, oc, in_c).bitcast(F32R))
                    for rh in range(rows // R):
                        ps = ps_mm()
                        for kc in range(in_c):
                            _mm(nc, ps[:], wt[:, kc, :],
                                in_t[:, kc, rh * R:(rh + 1) * R],
                                start=(kc == 0), stop=(kc == in_c - 1))
                        nc.scalar.activation(
                            out_t[:, oc, rh * R:(rh + 1) * R], ps[:],
                            func, bias=bias_s[:, oc:oc + 1])

            def vproj(copy_fn, src, w_v, vbias_s, wmpool):
                """Row-major V projection over all S rows in quarter blocks.

                copy_fn(rc, q, ps) stores the [P, 256] psum result for the
                256 output features of quarter q, key-row chunk rc."""
                for q in range(4):
                    wm = wmpool.tile([P, C, 256], F32R, tag="wmov",
                                     name=f"wm{q}")
                    nc.sync.dma_start(
                        out=wm[:],
                        in_=w_v.ap().rearrange("(k p) m -> p k m", p=P)
                        [:, :, q * 256:(q + 1) * 256].bitcast(F32R))
                    for rc in range(C):
                        ps = ps_mm(256)
                        for kc in range(C):
                            _mm(nc, ps[:], src[:, kc, rc * P:(rc + 1) * P],
                                wm[:, kc, :], start=(kc == 0),
                                stop=(kc == C - 1 and vbias_s is None))
                        if vbias_s is not None:
                            _mm(nc, ps[:], ones_row[:],
                                vbias_s[0:1, q * 256:(q + 1) * 256],
                                start=False, stop=True)
                        copy_fn(rc, q, ps)

            def layer_norm(a_t, n_c, g_s, b_s, out_t):
                """Row-wise LN over the (P * n_c) feature dim of a_t [P,n_c,R]."""
                inv_n = 1.0 / (n_c * P)
                ps_sum = psp.tile([1, R], F32, tag="score", bufs=2,
                                  name="lnsum")
                ps_sq = psp.tile([1, R], F32, tag="score", bufs=2,
                                 name="lnsumsq")
                for c in range(n_c):
                    sqc = tmp.tile([P, R], F32R, tag="lnsqc", name="lnsqc")
                    nc.vector.tensor_mul(sqc[:], a_t[:, c, :], a_t[:, c, :])
                    _mm(nc, ps_sum[:], ones_col[:], a_t[:, c, :],
                        start=(c == 0), stop=(c == n_c - 1))
                    _mm(nc, ps_sq[:], ones_col[:], sqc[:],
                        start=(c == 0), stop=(c == n_c - 1))
                mu = small.tile([1, R], F32R, tag="lnmu", name="lnmu")
                nc.scalar.activation(mu[:], ps_sum[:], AF.Copy, scale=inv_n)
                va = small.tile([1, R], F32, tag="lnva", name="lnva")
                nc.scalar.activation(va[:], ps_sq[:], AF.Copy, scale=inv_n)
                mu2 = small.tile([1, R], F32, tag="lnmu2", name="lnmu2")
                nc.vector.tensor_mul(mu2[:], mu[:], mu[:])
                nc.vector.tensor_sub(va[:], va[:], mu2[:])
                nc.scalar.activation(va[:], va[:], AF.Sqrt, bias=eps_t[:])
                rstd = small.tile([1, R], F32R, tag="lnrstd", name="lnrstd")
                nc.vector.reciprocal(rstd[:], va[:])
                nc.vector.tensor_mul(mu[:], mu[:], rstd[:])  # mu*rstd
                bca_ps = psp.tile([P, R], F32, tag="bcast", bufs=2,
                                  name="lnbca")
                _mm(nc, bca_ps[:], ones_row[:], rstd[:], start=True, stop=True)
                bcb_ps = psp.tile([P, R], F32, tag="bcast", bufs=2,
                                  name="lnbcb")
                _mm(nc, bcb_ps[:], ones_row[:], mu[:], start=True, stop=True)
                bca = tmp.tile([P, R], F32, tag="lnbcas", name="lnbcas")
                nc.scalar.copy(bca[:], bca_ps[:])
                bcb = tmp.tile([P, R], F32, tag="lnbcbs", name="lnbcbs")
                nc.scalar.copy(bcb[:], bcb_ps[:])
                for c in range(n_c):
                    nc.vector.tensor_mul(out_t[:, c, :], a_t[:, c, :], bca[:])
                    nc.vector.tensor_sub(out_t[:, c, :], out_t[:, c, :], bcb[:])
                    nc.vector.tensor_scalar(
                        out_t[:, c, :], out_t[:, c, :],
                        g_s[:, c:c + 1], b_s[:, c:c + 1], OP.mult, OP.add)

            def mha16(kv_src, q_src, w_q, w_k, w_v, bq_t, bk_t, bv_t, pools):
                """16-head attention; q over own R rows, k/v over S rows of
                kv_src.  Returns ctx feature-major [P, C, R]."""
                apool, vpool, kqpool, expool, wmpool = pools
                # V row-major with a ones column per head: [P, S/P, 16*65]
                v_sb = vpool.tile([P, C, H_SA * 65], F32R, tag="v_sa",
                                  name="v_sa")
                v4 = v_sb.rearrange("p r (h e) -> p r h e", e=65)
                nc.scalar.copy(
                    v4[:, :, :, 64],
                    ones_f[:, 0:H_SA * C].rearrange("p (r h) -> p r h", r=C))
                vproj(lambda rc, q, ps: nc.scalar.copy(
                          v4[:, rc, q * 4:(q + 1) * 4, 0:64],
                          ps.rearrange("p (h e) -> p h e", e=64)),
                      kv_src, w_v, bv_t, wmpool)
                ctx_t = apool.tile([P, C, R], F32R, tag="ctx_sa", name="ctx_sa")
                for oc in range(C):
                    # K chunk oc for all S rows; Q chunk oc for own R rows
                    wt = wpool.tile([P, C, P], F32R, tag="wstat",
                                    name=f"wstk{oc}")
                    nc.sync.dma_start(out=wt[:], in_=wstat_ap(w_k, oc, C).bitcast(F32R))
                    kf = kqpool.tile([P, S], F32R, tag="kf", name="kf")
                    for rh in range(2):
                        ps = ps_mm()
                        for kc in range(C):
                            _mm(nc, ps[:], wt[:, kc, :],
                                kv_src[:, kc, rh * R:(rh + 1) * R],
                                start=(kc == 0), stop=(kc == C - 1))
                        nc.scalar.activation(kf[:, rh * R:(rh + 1) * R], ps[:],
                                             AF.Identity, bias=bk_t[:, oc:oc + 1])
                    wtq = wpool.tile([P, C, P], F32R, tag="wstat",
                                     name=f"wstq{oc}")
                    nc.sync.dma_start(out=wtq[:], in_=wstat_ap(w_q, oc, C).bitcast(F32R))
                    qf = kqpool.tile([P, R], F32R, tag="qf", name="qf")
                    ps = ps_mm()
                    for kc in range(C):
                        _mm(nc, ps[:], wtq[:, kc, :], q_src[:, kc, 0:R],
                            start=(kc == 0), stop=(kc == C - 1))
                    nc.scalar.activation(qf[:], ps[:], AF.Identity,
                                         bias=bq_t[:, oc:oc + 1])
                    for hh in range(2):
                        h = oc * 2 + hh
                        po = hh * 64
                        ctx_ps = psp.tile([P, R], F32, tag="ctxps", bufs=2,
                                          name="ctxps")
                        for kc in range(C):
                            ps_s = psp.tile([P, R], F32, tag="score", bufs=2,
                                            name="score")
                            _mm(nc, ps_s[:], kf[po:po + 64, kc * P:(kc + 1) * P],
                                qf[po:po + 64, :], start=True, stop=True)
                            ex = expool.tile([P, R], F32R, tag="exp", name="ex")
                            nc.scalar.activation(ex[:], ps_s[:], AF.Exp,
                                                 scale=0.125)
                            _mm(nc, ctx_ps[:65, :], v4[:, kc, h, 0:65], ex[:],
                                start=(kc == 0), stop=(kc == C - 1))
                        rec = small.tile([1, R], F32R, tag="rec", bufs=2,
                                         name="rec")
                        nc.vector.reciprocal(rec[:], ctx_ps[64:65, :])
                        bc_ps = psp.tile([P, R], F32, tag="bcast", bufs=2,
                                         name="bcast")
                        _mm(nc, bc_ps[0:64, :], ones_row[0:1, 0:64], rec[:],
                            start=True, stop=True)
                        bc_sb = small.tile([64, R], F32, tag="bcsb", bufs=2,
                                           name="bcsb")
                        nc.scalar.copy(bc_sb[:], bc_ps[0:64, :])
                        nc.vector.tensor_mul(ctx_t[po:po + 64, oc, :],
                                             ctx_ps[0:64, :], bc_sb[:])
                return ctx_t

            # ---------------- phase 1: SGIRA self-attention ----------------
            ssp_cm = tc.tile_pool(name="ssp", bufs=1, side="right")
            ssp = ssp_cm.__enter__()
            ss = ssp.tile([P, C, R], F32R, tag="ss")
            with tc.tile_pool(name="p1", bufs=1) as p1, \
                 tc.tile_pool(name="p1kq", bufs=2) as p1kq, \
                 tc.tile_pool(name="p1ex", bufs=3) as p1ex, \
                 tc.tile_pool(name="p1wm", bufs=2) as p1wm:
                xT_s = p1.tile([P, C, S], F32R, tag="xT")
                nc.sync.dma_start(
                    out=xT_s[:],
                    in_=xT.ap().rearrange("(c p) r -> p c r", p=P)
                    .bitcast(F32R))
                ctx_sa = mha16(xT_s, xT_s, wq, wk, wv, bq_s, bk_s, bv_s,
                               (p1, p1, p1kq, p1ex, p1wm))
                # out-proj + residual + LN -> ss
                sa = p1.tile([P, C, R], F32R, tag="sa")
                proj(sa, C, ctx_sa, C, wo, bo_s, R)
                for c in range(C):
                    nc.vector.tensor_add(sa[:, c, :], sa[:, c, :],
                                         xT_s[:, c, 0:R])
                layer_norm(sa, C, nsg_s, nsb_s, ss)

            # ---------------- optional cross-attention (gate != 1) ---------
            if include_cross:
                fusedp_cm = tc.tile_pool(name="fusedp", bufs=1, side="right")
                fusedp = fusedp_cm.__enter__()
                fused = fusedp.tile([P, C, R], F32R, tag="fused")
                with tc.tile_pool(name="pc1", bufs=1) as pc1, \
                     tc.tile_pool(name="pc1kq", bufs=2) as pc1kq, \
                     tc.tile_pool(name="pc1ex", bufs=3) as pc1ex, \
                     tc.tile_pool(name="pc1wm", bufs=2) as pc1wm:
                    mT_s = pc1.tile([P, C, S], F32R, tag="mT")
                    nc.sync.dma_start(
                        out=mT_s[:],
                        in_=mT.ap().rearrange("(c p) r -> p c r", p=P)
                        .bitcast(F32R))
                    ctx_ca = mha16(mT_s, ss, cwq, cwk, cwv, cbq_s, cbk_s,
                                   cbv_s, (pc1, pc1, pc1kq, pc1ex, pc1wm))
                    ca = pc1.tile([P, C, R], F32R, tag="ca")
                    proj(ca, C, ctx_ca, C, cwo, cbo_s, R)
                    for c in range(C):
                        nc.vector.tensor_add(ca[:, c, :], ca[:, c, :],
                                             ss[:, c, :])
                    cs = pc1.tile([P, C, R], F32R, tag="cs")
                    layer_norm(ca, C, ncg_s, ncb_s, cs)
                    # fused = gate*ss + (1-gate)*cs
                    for c in range(C):
                        nc.vector.tensor_scalar(
                            fused[:, c, :], ss[:, c, :], gc_s[:, 0:1], None,
                            OP.mult)
                        nc.vector.tensor_scalar(
                            cs[:, c, :], cs[:, c, :], g1_s[:, 0:1], None,
                            OP.mult)
                        nc.vector.tensor_add(fused[:, c, :], fused[:, c, :],
                                             cs[:, c, :])
                ff_in = fused
            else:
                ff_in = ss

            # ---------------- phase 2: FFN ----------------
            hidp_cm = tc.tile_pool(name="hidp", bufs=1)
            hidp = hidp_cm.__enter__()
            hidden = hidp.tile([P, C, R], F32R, tag="hidden")
            with tc.tile_pool(name="p2", bufs=1) as p2:
                h1 = p2.tile([P, CFF, R], F32R, tag="h1")
                proj(h1, CFF, ff_in, C, w1, b1_s, R, func=AF.Gelu)
                ffo = p2.tile([P, C, R], F32R, tag="ffo")
                for oc in range(C):
                    wt2 = p2.tile([P, CFF, P], F32R, tag="wstat2", bufs=2,
                                  name=f"wst2_{oc}")
                    nc.sync.dma_start(out=wt2[:],
                                      in_=wstat_ap(w2, oc, CFF).bitcast(F32R))
                    ps = ps_mm()
                    for kc in range(CFF):
                        _mm(nc, ps[:], wt2[:, kc, :], h1[:, kc, :],
                            start=(kc == 0), stop=(kc == CFF - 1))
                    nc.scalar.activation(ffo[:, oc, :], ps[:], AF.Identity,
                                         bias=b2_s[:, oc:oc + 1])
                for c in range(C):
                    nc.vector.tensor_add(ffo[:, c, :], ffo[:, c, :],
                                         ff_in[:, c, :])
                layer_norm(ffo, C, nfg_s, nfb_s, hidden)
            # ss (or fused) no longer needed
            if include_cross:
                fusedp_cm.__exit__(None, None, None)
            ssp_cm.__exit__(None, None, None)

            # ---------------- phase 3: SAIGA squeeze-excite ----------------
            sep_cm = tc.tile_pool(name="sep", bufs=1, side="right")
            sep = sep_cm.__enter__()
            se_own = sep.tile([P, C, R], F32R, tag="se_own")
            with tc.tile_pool(name="p3", bufs=1) as p3:
                h2 = p3.tile([P, C2, R], F32R, tag="h2")
                proj(h2, C2, hidden, C, exw, exb_s, R, func=AF.Relu)
                sqo = p3.tile([P, C, R], F32R, tag="sqo")
                proj(sqo, C, h2, C2, sqw, sqb_s, R)
                for c in range(C):
                    nc.vector.tensor_add(sqo[:, c, :], sqo[:, c, :],
                                         hidden[:, c, :])
                layer_norm(sqo, C, nrg_s, nrb_s, se_own)
            hidp_cm.__exit__(None, None, None)

            # ------- phase 4: pairwise AllGather of se; phase 5: SAIGA -----
            with tc.tile_pool(name="p5", bufs=1) as p5, \
                 tc.tile_pool(name="p5kq", bufs=2) as p5kq, \
                 tc.tile_pool(name="p5ex", bufs=3) as p5ex, \
                 tc.tile_pool(name="p5wm", bufs=2) as p5wm, \
                 tc.tile_pool(name="dramp", bufs=1, space="DRAM") as dramp:
                in_b = dramp.tile([D, R], F32, tag="cc_in")
                gat = dramp.tile([2, D, R], F32, tag="cc_out")
                nc.gpsimd.dma_start(
                    out=in_b.rearrange("(c p) r -> p c r", p=P),
                    in_=se_own[:].bitcast(F32))
                nc.gpsimd.collective_compute(
                    "AllGather", OP.bypass,
                    replica_groups=[[0, 1], [2, 3], [4, 5], [6, 7]],
                    ins=[in_b.opt()], outs=[gat.opt()])
                se_full = p5.tile([P, C, S], F32R, tag="se_full")
                for r in range(2):
                    nc.sync.dma_start(
                        out=se_full[:, :, r * R:(r + 1) * R],
                        in_=gat[r].rearrange("(c p) r -> p c r", p=P)
                        .bitcast(F32R))

                # V2 row-major [P, S/P, 1024] (head hd=256)
                v2 = p5.tile([P, C, D], F32R, tag="v2")
                vproj(lambda rc, q, ps: nc.scalar.copy(
                          v2[:, rc, q * 256:(q + 1) * 256], ps[:]),
                      se_full, vw, vb_s, p5wm)

                ctx2 = p5.tile([P, C, R], F32R, tag="ctx2")
                for h in range(H_SG):
                    k2 = []
                    q2 = []
                    for i in range(2):
                        oc = 2 * h + i
                        wt = wpool.tile([P, C, P], F32R, tag="wstat",
                                        name=f"wstk2_{oc}")
                        nc.sync.dma_start(out=wt[:], in_=wstat_ap(kw, oc, C).bitcast(F32R))
                        kt = p5kq.tile([P, S], F32R, tag="k2", name=f"k2_{i}")
                        for rh in range(2):
                            ps = ps_mm()
                            for kc in range(C):
                                _mm(nc, ps[:], wt[:, kc, :],
                                    se_full[:, kc, rh * R:(rh + 1) * R],
                                    start=(kc == 0), stop=(kc == C - 1))
                            nc.scalar.activation(kt[:, rh * R:(rh + 1) * R],
                                                 ps[:], AF.Identity,
                                                 bias=kb_s[:, oc:oc + 1])
                        k2.append(kt)
                        wtq = wpool.tile([P, C, P], F32R, tag="wstat",
                                         name=f"wstq2_{oc}")
                        nc.sync.dma_start(out=wtq[:], in_=wstat_ap(qw, oc, C).bitcast(F32R))
                        qt = p5kq.tile([P, R], F32R, tag="q2", name=f"q2_{i}")
                        ps = ps_mm()
                        for kc in range(C):
                            _mm(nc, ps[:], wtq[:, kc, :], se_own[:, kc, :],
                                start=(kc == 0), stop=(kc == C - 1))
                        nc.scalar.activation(qt[:], ps[:], AF.Identity,
                                             bias=qb_s[:, oc:oc + 1])
                        q2.append(qt)
                    ctx_ps = [psp.tile([P, R], F32, tag="ctxps", bufs=2,
                                       name=f"ctxps{mh}")
                              for mh in range(2)]
                    sum_ps = psp.tile([1, R], F32, tag="bcast", bufs=2,
                                      name="asum")
                    for kc in range(C):
                        ps_s = psp.tile([P, R], F32, tag="score", bufs=2,
                                        name="score2")
                        _mm(nc, ps_s[:], k2[0][:, kc * P:(kc + 1) * P],
                            q2[0][:], start=True, stop=False)
                        _mm(nc, ps_s[:], k2[1][:, kc * P:(kc + 1) * P],
                            q2[1][:], start=False, stop=True)
                        ex = p5ex.tile([P, R], F32R, tag="exp", name="ex2")
                        nc.scalar.activation(ex[:], ps_s[:], AF.Exp,
                                             scale=0.0625)
                        _mm(nc, sum_ps[:], ones_col[:], ex[:],
                            start=(kc == 0), stop=(kc == C - 1))
                        for mh in range(2):
                            _mm(nc, ctx_ps[mh][:],
                                v2[:, kc, (h * 256 + mh * P):(h * 256 + (mh + 1) * P)],
                                ex[:], start=(kc == 0), stop=(kc == C - 1))
                    rec = small.tile([1, R], F32R, tag="rec", bufs=2,
                                     name="rec2")
                    nc.vector.reciprocal(rec[:], sum_ps[:])
                    bc_ps = psp.tile([P, R], F32, tag="bcast", bufs=2,
                                     name="bcast2")
                    _mm(nc, bc_ps[:], ones_row[:], rec[:], start=True,
                        stop=True)
                    bc_sb = tmp.tile([P, R], F32, tag="bcsb2", name="bcsb2")
                    nc.scalar.copy(bc_sb[:], bc_ps[:])
                    for mh in range(2):
                        nc.vector.tensor_mul(ctx2[:, 2 * h + mh, :],
                                             ctx_ps[mh][:], bc_sb[:])

                # ---------------- phase 6: final residual + LN -------------
                for c in range(C):
                    nc.vector.tensor_add(ctx2[:, c, :], ctx2[:, c, :],
                                         se_own[:, c, :])
                fin = p5.tile([P, C, R], F32, tag="fin")
                layer_norm(ctx2, C, nrg_s, nrb_s, fin)
                nc.sync.dma_start(
                    out=out_d.ap().rearrange("(c p) r -> p c r", p=P),
                    in_=fin[:])
            sep_cm.__exit__(None, None, None)

    nc.compile()
    return nc


def _pc(v):
    """[n*128] -> [128, n] per-partition layout."""
    v = np.asarray(v, np.float32)
    return np.ascontiguousarray(v.reshape(-1, P).T)


def kernel(**inputs):
    x = np.asarray(inputs["input_states"], np.float32)
    gate = float(np.asarray(inputs["gate"]).ravel()[0])
    include_cross = (gate != 1.0)

    bq, bk, bv = np.split(np.asarray(inputs["sa_in_b"], np.float32), 3)
    vb = np.asarray(inputs["v_b"], np.float32)
    cbv = (np.split(np.asarray(inputs["ca_in_b"], np.float32), 3)[2]
           if include_cross else np.zeros(1, np.float32))
    with_vbias = bool(np.any(bv) or np.any(vb) or np.any(cbv))

    key = (include_cross, with_vbias)
    if key not in _CACHE:
        _CACHE[key] = _build(include_cross, with_vbias)
    nc = _CACHE[key]

    wq, wk, wv = [np.ascontiguousarray(w) for w in
                  np.split(np.asarray(inputs["sa_in_w"], np.float32), 3, axis=1)]

    shared = {
        "wq": wq, "wk": wk, "wv": wv,
        "bq": _pc(bq), "bk": _pc(bk),
        "wo": np.ascontiguousarray(np.asarray(inputs["sa_out_w"], np.float32)),
        "bo": _pc(inputs["sa_out_b"]),
        "w1": np.ascontiguousarray(np.asarray(inputs["ffn_w1"], np.float32)),
        "b1": _pc(inputs["ffn_b1"]),
        "w2": np.ascontiguousarray(np.asarray(inputs["ffn_w2"], np.float32)),
        "b2": _pc(inputs["ffn_b2"]),
        "exw": np.ascontiguousarray(np.asarray(inputs["ex_w"], np.float32)),
        "exb": _pc(inputs["ex_b"]),
        "sqw": np.ascontiguousarray(np.asarray(inputs["sq_w"], np.float32)),
        "sqb": _pc(inputs["sq_b"]),
        "qw": np.ascontiguousarray(np.asarray(inputs["q_w"], np.float32)),
        "qb": _pc(inputs["q_b"]),
        "kw": np.ascontiguousarray(np.asarray(inputs["k_w"], np.float32)),
        "kb": _pc(inputs["k_b"]),
        "vw": np.ascontiguousarray(np.asarray(inputs["v_w"], np.float32)),
        "nsg": _pc(inputs["ns_g"]), "nsb": _pc(inputs["ns_b"]),
        "nfg": _pc(inputs["nf_g"]), "nfb": _pc(inputs["nf_b"]),
        "nrg": _pc(inputs["nrm_g"]), "nrb": _pc(inputs["nrm_b"]),
    }
    if with_vbias:
        shared["bv"] = np.ascontiguousarray(bv.reshape(1, D))
        shared["vb"] = np.ascontiguousarray(vb.reshape(1, D))
    if include_cross:
        m = np.asarray(inputs["memory_states"], np.float32)
        cwq, cwk, cwv = [np.ascontiguousarray(w) for w in
                         np.split(np.asarray(inputs["ca_in_w"], np.float32),
                                  3, axis=1)]
        cbq, cbk, cbv_ = np.split(np.asarray(inputs["ca_in_b"], np.float32), 3)
        shared.update({
            "cwq": cwq, "cwk": cwk, "cwv": cwv,
            "cbq": _pc(cbq), "cbk": _pc(cbk),
            "cwo": np.ascontiguousarray(
                np.asarray(inputs["ca_out_w"], np.float32)),
            "cbo": _pc(inputs["ca_out_b"]),
            "ncg": _pc(inputs["nc_g"]), "ncb": _pc(inputs["nc_b"]),
            "gate_c": np.full((P, 1), gate, np.float32),
            "gate_1mc": np.full((P, 1), 1.0 - gate, np.float32),
        })
        if with_vbias:
            shared["cbv"] = np.ascontiguousarray(cbv_.reshape(1, D))

    in_maps = []
    for c in range(N_CORES):
        b, hf = c // 2, c % 2
        xp = np.concatenate([x[b, hf * R:(hf + 1) * R],
                             x[b, (1 - hf) * R:(2 - hf) * R]], axis=0)
        m_in = dict(shared)
        m_in["xT"] = np.ascontiguousarray(xp.T)
        if include_cross:
            m_in["mT"] = np.ascontiguousarray(m[b].T)
        in_maps.append(m_in)

    res = bass_utils.run_bass_kernel_spmd(nc, in_maps,
                                          core_ids=list(range(N_CORES)))
    out = np.empty((4, S, D), np.float32)
    for c in range(N_CORES):
        b, hf = c // 2, c % 2
        out[b, hf * R:(hf + 1) * R, :] = res.results[c]["out"].T
    return out



# revision 6
# speedup vs baseline: 1.2132x; 1.2132x over previous
"""Trainium2 Bass kernel for nn_LinearTransformerLayer_44495861187342.

Reference network: SGIRA block (self-attn MHA-16h -> LN -> cross-attn -> LN ->
gate blend -> FFN(gelu) -> LN) followed by a SAIGA block (squeeze-excite MLP ->
LN -> 4-head self-attn -> LN).  With the shipped inputs gate == 1.0, so the
cross-attention branch is algebraically dead and memory_states is unused; a
general path that includes it is kept for gate != 1.

Sharding (8 NeuronCores): core c owns 512 rows = (batch c//2, half c%2) of the
[4, 1024, 1024] input.  Row-local ops (projections, FFN, layernorm, softmax)
shard perfectly.  SGIRA self-attention K/V are recomputed per core for the full
1024-row batch (input replicated host-side).  SAIGA K/V are computed for the
own 512 rows only and exchanged within each core pair via one AllReduce(add);
the peer half is recovered as sum - own, which overlaps the collective with the
own-half attention work.

Layout: activations are feature-major in SBUF ([feat partitions, rows free]) so
matmuls contract the partition dim against natural-layout weights, and
layernorm/softmax feature reductions are PE ones-matmuls.  All matmul operands
are bf16 (fp32 accumulation in PSUM); LN statistics and the final output are
fp32.  Scalar engine handles exp/gelu; all other PSUM->SBUF copies run on the
vector engine with the bias folded in.
"""

import contextlib

import ml_dtypes
import numpy as np

import concourse.bass as bass
import concourse.mybir as mybir
import concourse.tile as tile
from concourse import bacc
from concourse import bass_utils

F32 = mybir.dt.float32
BF16 = mybir.dt.bfloat16
AF = mybir.ActivationFunctionType
OP = mybir.AluOpType

D = 1024          # model dim
DFF = 4096        # ffn dim
D2 = 2048         # squeeze-excite dim
S = 1024          # full sequence rows per batch
R = 512           # rows owned per core
P = 128
C = D // P        # 8 feature chunks
CFF = DFF // P    # 32
C2 = D2 // P      # 16
H_SA = 16         # SGIRA heads (hd 64)
H_SG = 4          # SAIGA heads (hd 256)
N_CORES = 8
EPS = 1e-5

_CACHE = {}


def _build(include_cross: bool, with_vbias: bool):
    nc = bacc.Bacc("TRN2", target_bir_lowering=False, debug=False,
                   num_devices=N_CORES)

    def din(name, shape, dt=BF16):
        return nc.dram_tensor(name, shape, dt, kind="ExternalInput")

    # feature-major inputs (host pre-transposed), own 512 rows first
    xT = din("xT", [D, S])
    wq = din("wq", [D, D]); wk = din("wk", [D, D]); wv = din("wv", [D, D])
    bq = din("bq", [P, C], F32); bk = din("bk", [P, C], F32)
    wo = din("wo", [D, D]); bo = din("bo", [P, C], F32)
    w1 = din("w1", [D, DFF]); b1 = din("b1", [P, CFF], F32)
    w2 = din("w2", [DFF, D]); b2 = din("b2", [P, C], F32)
    exw = din("exw", [D, D2]); exb = din("exb", [P, C2], F32)
    sqw = din("sqw", [D2, D]); sqb = din("sqb", [P, C], F32)
    qw = din("qw", [D, D]); qb = din("qb", [P, C], F32)
    kw = din("kw", [D, D]); kb = din("kb", [P, C], F32)
    vw = din("vw", [D, D])
    nsg = din("nsg", [P, C], F32); nsb = din("nsb", [P, C], F32)
    nfg = din("nfg", [P, C], F32); nfb = din("nfb", [P, C], F32)
    nrg = din("nrg", [P, C], F32); nrb = din("nrb", [P, C], F32)
    if with_vbias:
        bv = din("bv", [1, D])
        vb = din("vb", [1, D])
    if include_cross:
        mT = din("mT", [D, S])
        cwq = din("cwq", [D, D]); cwk = din("cwk", [D, D]); cwv = din("cwv", [D, D])
        cbq = din("cbq", [P, C], F32); cbk = din("cbk", [P, C], F32)
        cwo = din("cwo", [D, D]); cbo = din("cbo", [P, C], F32)
        ncg = din("ncg", [P, C], F32); ncb = din("ncb", [P, C], F32)
        gate_c = din("gate_c", [P, 1], F32)      # broadcast gate
        gate_1mc = din("gate_1mc", [P, 1], F32)  # broadcast (1 - gate)
        if with_vbias:
            cbv = din("cbv", [1, D])

    out_d = nc.dram_tensor("out", [D, R], F32, kind="ExternalOutput")

    with tile.TileContext(nc) as tc:
        with contextlib.ExitStack() as ctx, \
             nc.allow_low_precision("bf16 operands feeding the PE"):
            const = ctx.enter_context(tc.tile_pool(name="const", bufs=1))
            wpool = ctx.enter_context(tc.tile_pool(name="wpool", bufs=4))
            # PSUM: mm 2 + score 4 + ctx 2 = 8 banks
            psp = ctx.enter_context(tc.tile_pool(name="psp", bufs=1,
                                                 space="PSUM"))
            small = ctx.enter_context(tc.tile_pool(name="small", bufs=2))

            def ps_mm():
                return psp.tile([P, R], F32, tag="mm", bufs=2, name="psmm")

            def ps_score(name="score"):
                return psp.tile([P, R], F32, tag="score", bufs=4, name=name)

            def ps_ctx(name="ctxps"):
                return psp.tile([P, R], F32, tag="ctx", bufs=2, name=name)

            ones_col = const.tile([P, 1], BF16, tag="ones_col")
            nc.vector.memset(ones_col[:], 1.0)
            ones_row = const.tile([1, P], BF16, tag="ones_row")
            nc.vector.memset(ones_row[:], 1.0)
            eps_t = const.tile([1, 1], F32, tag="eps")
            nc.vector.memset(eps_t[:], EPS)

            def load_pc(dram):  # small per-partition tables
                t = const.tile(list(dram.shape), dram.dtype,
                               tag=dram.name + "_sb")
                nc.sync.dma_start(out=t[:], in_=dram.ap())
                return t

            def wstat_ap(w, oc, kcs):
                # [P, kcs, 128] stationary block: w[(kc p), oc*128 + m]
                return w.ap().rearrange("(k p) m -> p k m", p=P)[:, :, oc * P:(oc + 1) * P]

            def copy_bias(out_ap, ps, bias_s):
                """PSUM -> SBUF copy with per-partition bias on the DVE."""
                nc.vector.tensor_scalar(out_ap, ps, bias_s, None, OP.add)

            def proj(out_t, out_c, in_t, in_c, w, bias_s, func=None):
                """out_t[:, oc, r] = func(sum_k w[k, oc*128+p] * in_t[k, r] + b)

                over the own R rows of in_t."""
                for oc in range(out_c):
                    wt = wpool.tile([P, in_c, P], BF16, tag="wstat",
                                    name=f"wst_{w.name}{oc}")
                    nc.sync.dma_start(out=wt[:], in_=wstat_ap(w, oc, in_c))
                    ps = ps_mm()
                    for kc in range(in_c):
                        nc.tensor.matmul(ps[:], wt[:, kc, :], in_t[:, kc, 0:R],
                                         start=(kc == 0), stop=(kc == in_c - 1))
                    if func == "gelu":
                        nc.scalar.activation(out_t[:, oc, :], ps[:], AF.Gelu,
                                             bias=bias_s[:, oc:oc + 1])
                    elif func == "relu":
                        nc.vector.tensor_scalar(out_t[:, oc, :], ps[:],
                                                bias_s[:, oc:oc + 1], 0.0,
                                                OP.add, OP.max)
                    else:
                        copy_bias(out_t[:, oc, :], ps[:], bias_s[:, oc:oc + 1])

            def layer_norm(a_t, n_c, g_s, b_s, out_t):
                """Row-wise LN over the (P * n_c) feature dim of a_t [P,n_c,R]."""
                inv_n = 1.0 / (n_c * P)
                ps_sum = ps_score("lnsum")
                ps_sq = ps_score("lnsumsq")
                for c in range(n_c):
                    sqc = small.tile([P, R], BF16, tag="lnsqc", name="lnsqc")
                    nc.vector.tensor_mul(sqc[:], a_t[:, c, :], a_t[:, c, :])
                    nc.tensor.matmul(ps_sum[0:1, :], ones_col[:], a_t[:, c, :],
                                     start=(c == 0), stop=(c == n_c - 1))
                    nc.tensor.matmul(ps_sq[0:1, :], ones_col[:], sqc[:],
                                     start=(c == 0), stop=(c == n_c - 1))
                mu = small.tile([1, R], BF16, tag="lnmu", name="lnmu")
                nc.scalar.activation(mu[:], ps_sum[0:1, :], AF.Copy,
                                     scale=inv_n)
                va = small.tile([1, R], F32, tag="lnva", name="lnva")
                nc.scalar.activation(va[:], ps_sq[0:1, :], AF.Copy,
                                     scale=inv_n)
                mu2 = small.tile([1, R], F32, tag="lnmu2", name="lnmu2")
                nc.vector.tensor_mul(mu2[:], mu[:], mu[:])
                nc.vector.tensor_sub(va[:], va[:], mu2[:])
                nc.scalar.activation(va[:], va[:], AF.Sqrt, bias=eps_t[:])
                rstd = small.tile([1, R], BF16, tag="lnrstd", name="lnrstd")
                nc.vector.reciprocal(rstd[:], va[:])
                nc.vector.tensor_mul(mu[:], mu[:], rstd[:])  # mu*rstd
                bca_ps = ps_score("lnbca")
                nc.tensor.matmul(bca_ps[:], ones_row[:], rstd[:],
                                 start=True, stop=True)
                bcb_ps = ps_score("lnbcb")
                nc.tensor.matmul(bcb_ps[:], ones_row[:], mu[0:1, :],
                                 start=True, stop=True)
                bca = small.tile([P, R], BF16, tag="lnbcas", name="lnbcas")
                nc.vector.tensor_scalar(bca[:], bca_ps[:], 0.0, None, OP.add)
                bcb = small.tile([P, R], BF16, tag="lnbcbs", name="lnbcbs")
                nc.vector.tensor_scalar(bcb[:], bcb_ps[:], 0.0, None, OP.add)
                for c in range(n_c):
                    nc.vector.tensor_mul(out_t[:, c, :], a_t[:, c, :], bca[:])
                    nc.vector.tensor_sub(out_t[:, c, :], out_t[:, c, :],
                                         bcb[:])
                    nc.vector.tensor_scalar(
                        out_t[:, c, :], out_t[:, c, :],
                        g_s[:, c:c + 1], b_s[:, c:c + 1], OP.mult, OP.add)

            def vproj_16(kv_src, w_v, vbias_t, vpool, wmpool, rcs, tag):
                """Row-major V for 16-head attention: [P, rc, 16, 65] with a
                ones column per head (softmax denominator trick)."""
                v4 = vpool.tile([P, len(rcs), H_SA, 65], BF16, tag=tag)
                nc.vector.memset(v4[:, :, :, 64:65], 1.0)
                for qh in range(2):
                    wm = wmpool.tile([P, C, R], BF16, tag="wmov",
                                     name=f"wm_{tag}{qh}")
                    nc.sync.dma_start(
                        out=wm[:],
                        in_=w_v.ap().rearrange("(k p) m -> p k m", p=P)
                        [:, :, qh * R:(qh + 1) * R])
                    for i, rc in enumerate(rcs):
                        ps = ps_mm()
                        for kc in range(C):
                            nc.tensor.matmul(
                                ps[:], kv_src[:, kc, rc * P:(rc + 1) * P],
                                wm[:, kc, :], start=(kc == 0),
                                stop=(kc == C - 1 and vbias_t is None))
                        if vbias_t is not None:
                            nc.tensor.matmul(
                                ps[:], ones_row[:],
                                vbias_t[0:1, qh * R:(qh + 1) * R],
                                start=False, stop=True)
                        nc.vector.tensor_scalar(
                            v4[:, i, qh * 8:(qh + 1) * 8, 0:64],
                            ps.rearrange("p (h e) -> p h e", e=64),
                            0.0, None, OP.add)
                return v4

            def mha16(kv_src, q_src, w_q, w_k, w_v, bq_t, bk_t, bv_t, pools):
                """16-head attention; q over own R rows, k/v over S rows of
                kv_src.  Returns ctx feature-major [P, C, R]."""
                apool, vpool, kqpool, expool, wmpool = pools
                v4 = vproj_16(kv_src, w_v, bv_t, vpool, wmpool,
                              list(range(C)), "v_sa")
                ctx_t = apool.tile([P, C, R], BF16, tag="ctx_sa",
                                   name="ctx_sa")
                kq = {}

                def kq_proj(oc):
                    wt = wpool.tile([P, C, P], BF16, tag="wstat",
                                    name=f"wstk{oc}")
                    nc.sync.dma_start(out=wt[:], in_=wstat_ap(w_k, oc, C))
                    kf = kqpool.tile([P, S], BF16, tag="kf", name="kf")
                    for rh in range(2):
                        ps = ps_mm()
                        for kc in range(C):
                            nc.tensor.matmul(
                                ps[:], wt[:, kc, :],
                                kv_src[:, kc, rh * R:(rh + 1) * R],
                                start=(kc == 0), stop=(kc == C - 1))
                        copy_bias(kf[:, rh * R:(rh + 1) * R], ps[:],
                                  bk_t[:, oc:oc + 1])
                    wtq = wpool.tile([P, C, P], BF16, tag="wstat",
                                     name=f"wstq{oc}")
                    nc.sync.dma_start(out=wtq[:], in_=wstat_ap(w_q, oc, C))
                    qf = kqpool.tile([P, R], BF16, tag="qf", name="qf")
                    ps = ps_mm()
                    for kc in range(C):
                        nc.tensor.matmul(ps[:], wtq[:, kc, :],
                                         q_src[:, kc, 0:R],
                                         start=(kc == 0), stop=(kc == C - 1))
                    copy_bias(qf[:], ps[:], bq_t[:, oc:oc + 1])
                    kq[oc] = (kf, qf)

                def att(oc):
                    kf, qf = kq.pop(oc)
                    ctx_ps = [ps_ctx(f"ctxps{hh}") for hh in range(2)]
                    ex = {}
                    for kc in range(C):
                        # row-tiled concurrent score pair (heads 2oc, 2oc+1)
                        for hh in range(2):
                            po = hh * 64
                            ps_s = ps_score(f"score{hh}")
                            nc.tensor.matmul(
                                ps_s[:], kf[po:po + 64, kc * P:(kc + 1) * P],
                                qf[po:po + 64, :], start=True, stop=True)
                            ex[hh] = expool.tile([P, R], BF16, tag="exp",
                                                 name=f"ex{hh}")
                            nc.scalar.activation(ex[hh][:], ps_s[:], AF.Exp,
                                                 scale=0.125)
                        for hh in range(2):
                            h = oc * 2 + hh
                            nc.tensor.matmul(
                                ctx_ps[hh][:65, :],
                                v4[:, kc, h, 0:65], ex[hh][:],
                                start=(kc == 0), stop=(kc == C - 1))
                    for hh in range(2):
                        po = hh * 64
                        rec = small.tile([1, R], BF16, tag="rec", name="rec")
                        nc.vector.reciprocal(rec[:], ctx_ps[hh][64:65, :])
                        bc_ps = ps_score("bcast")
                        nc.tensor.matmul(bc_ps[0:64, :], ones_row[0:1, 0:64],
                                         rec[:], start=True, stop=True)
                        bc_sb = small.tile([64, R], BF16, tag="bcsb",
                                           name="bcsb")
                        nc.vector.tensor_scalar(bc_sb[:], bc_ps[0:64, :],
                                                0.0, None, OP.add)
                        nc.vector.tensor_mul(ctx_t[po:po + 64, oc, :],
                                             ctx_ps[hh][0:64, :], bc_sb[:])

                kq_proj(0)
                kq_proj(1)
                for oc in range(C):
                    if oc + 2 < C:
                        kq_proj(oc + 2)
                    att(oc)
                return ctx_t

            # ---------------- phase 1: SGIRA self-attention ----------------
            ssp_cm = tc.tile_pool(name="ssp", bufs=1, side="right")
            ssp = ssp_cm.__enter__()
            ss = ssp.tile([P, C, R], BF16, tag="ss")
            with tc.tile_pool(name="p1", bufs=1) as p1, \
                 tc.tile_pool(name="p1kq", bufs=2) as p1kq, \
                 tc.tile_pool(name="p1ex", bufs=3) as p1ex, \
                 tc.tile_pool(name="p1wm", bufs=2) as p1wm:
                xT_s = p1.tile([P, C, S], BF16, tag="xT")
                xt_ap = xT.ap().rearrange("(c p) r -> p c r", p=P)
                for c in range(C):
                    nc.sync.dma_start(out=xT_s[:, c, :], in_=xt_ap[:, c, :])
                bq_s = load_pc(bq); bk_s = load_pc(bk)
                bv_s = load_pc(bv) if with_vbias else None
                ctx_sa = mha16(xT_s, xT_s, wq, wk, wv, bq_s, bk_s, bv_s,
                               (p1, p1, p1kq, p1ex, p1wm))
                # out-proj + residual + LN -> ss
                bo_s = load_pc(bo)
                nsg_s = load_pc(nsg); nsb_s = load_pc(nsb)
                sa = p1.tile([P, C, R], BF16, tag="sa")
                proj(sa, C, ctx_sa, C, wo, bo_s)
                for c in range(C):
                    nc.vector.tensor_add(sa[:, c, :], sa[:, c, :],
                                         xT_s[:, c, 0:R])
                layer_norm(sa, C, nsg_s, nsb_s, ss)

            # ---------------- optional cross-attention (gate != 1) ---------
            if include_cross:
                fusedp_cm = tc.tile_pool(name="fusedp", bufs=1, side="right")
                fusedp = fusedp_cm.__enter__()
                fused = fusedp.tile([P, C, R], BF16, tag="fused")
                with tc.tile_pool(name="pc1", bufs=1) as pc1, \
                     tc.tile_pool(name="pc1kq", bufs=2) as pc1kq, \
                     tc.tile_pool(name="pc1ex", bufs=3) as pc1ex, \
                     tc.tile_pool(name="pc1wm", bufs=2) as pc1wm:
                    mT_s = pc1.tile([P, C, S], BF16, tag="mT")
                    mt_ap = mT.ap().rearrange("(c p) r -> p c r", p=P)
                    for c in range(C):
                        nc.sync.dma_start(out=mT_s[:, c, :], in_=mt_ap[:, c, :])
                    cbq_s = load_pc(cbq); cbk_s = load_pc(cbk)
                    cbv_s = load_pc(cbv) if with_vbias else None
                    ctx_ca = mha16(mT_s, ss, cwq, cwk, cwv, cbq_s, cbk_s,
                                   cbv_s, (pc1, pc1, pc1kq, pc1ex, pc1wm))
                    cbo_s = load_pc(cbo)
                    ncg_s = load_pc(ncg); ncb_s = load_pc(ncb)
                    gc_s = load_pc(gate_c); g1_s = load_pc(gate_1mc)
                    ca = pc1.tile([P, C, R], BF16, tag="ca")
                    proj(ca, C, ctx_ca, C, cwo, cbo_s)
                    for c in range(C):
                        nc.vector.tensor_add(ca[:, c, :], ca[:, c, :],
                                             ss[:, c, :])
                    cs = pc1.tile([P, C, R], BF16, tag="cs")
                    layer_norm(ca, C, ncg_s, ncb_s, cs)
                    # fused = gate*ss + (1-gate)*cs
                    for c in range(C):
                        nc.vector.tensor_scalar(
                            fused[:, c, :], ss[:, c, :], gc_s[:, 0:1], None,
                            OP.mult)
                        nc.vector.tensor_scalar(
                            cs[:, c, :], cs[:, c, :], g1_s[:, 0:1], None,
                            OP.mult)
                        nc.vector.tensor_add(fused[:, c, :], fused[:, c, :],
                                             cs[:, c, :])
                ff_in = fused
            else:
                ff_in = ss

            # ---------------- phase 2: FFN ----------------
            hidp_cm = tc.tile_pool(name="hidp", bufs=1)
            hidp = hidp_cm.__enter__()
            hidden = hidp.tile([P, C, R], BF16, tag="hidden")
            with tc.tile_pool(name="p2", bufs=1) as p2:
                b1_s = load_pc(b1); b2_s = load_pc(b2)
                nfg_s = load_pc(nfg); nfb_s = load_pc(nfb)
                h1 = p2.tile([P, CFF, R], BF16, tag="h1")
                proj(h1, CFF, ff_in, C, w1, b1_s, func="gelu")
                ffo = p2.tile([P, C, R], BF16, tag="ffo")
                for oc in range(C):
                    wt2 = p2.tile([P, CFF, P], BF16, tag="wstat2", bufs=2,
                                  name=f"wst2_{oc}")
                    nc.sync.dma_start(out=wt2[:], in_=wstat_ap(w2, oc, CFF))
                    ps = ps_mm()
                    for kc in range(CFF):
                        nc.tensor.matmul(ps[:], wt2[:, kc, :], h1[:, kc, :],
                                         start=(kc == 0), stop=(kc == CFF - 1))
                    copy_bias(ffo[:, oc, :], ps[:], b2_s[:, oc:oc + 1])
                for c in range(C):
                    nc.vector.tensor_add(ffo[:, c, :], ffo[:, c, :],
                                         ff_in[:, c, :])
                layer_norm(ffo, C, nfg_s, nfb_s, hidden)
            # ss (or fused) no longer needed
            if include_cross:
                fusedp_cm.__exit__(None, None, None)
            ssp_cm.__exit__(None, None, None)

            # ---------------- phase 3: SAIGA squeeze-excite ----------------
            sep_cm = tc.tile_pool(name="sep", bufs=1, side="right")
            sep = sep_cm.__enter__()
            se_own = sep.tile([P, C, R], BF16, tag="se_own")
            with tc.tile_pool(name="p3", bufs=1) as p3:
                exb_s = load_pc(exb); sqb_s = load_pc(sqb)
                nrg_s = load_pc(nrg); nrb_s = load_pc(nrb)
                h2 = p3.tile([P, C2, R], BF16, tag="h2")
                proj(h2, C2, hidden, C, exw, exb_s, func="relu")
                sqo = p3.tile([P, C, R], BF16, tag="sqo")
                proj(sqo, C, h2, C2, sqw, sqb_s)
                for c in range(C):
                    nc.vector.tensor_add(sqo[:, c, :], sqo[:, c, :],
                                         hidden[:, c, :])
                layer_norm(sqo, C, nrg_s, nrb_s, se_own)
            hidp_cm.__exit__(None, None, None)

            # ------- phase 4: SAIGA K/V own-half + pair AllReduce ----------
            # ------- phase 5: 4-head attention (own half overlaps the cc) --
            with tc.tile_pool(name="p5", bufs=1) as p5, \
                 tc.tile_pool(name="p5ex", bufs=3) as p5ex, \
                 tc.tile_pool(name="p5wm", bufs=2) as p5wm, \
                 tc.tile_pool(name="dramp", bufs=1, space="DRAM") as dramp:
                qb_s = load_pc(qb); kb_s = load_pc(kb)
                vb_s = load_pc(vb) if with_vbias else None

                # K2 feature-major [P, C, S]; own rows first
                k2 = p5.tile([P, C, S], BF16, tag="k2")
                for oc in range(C):
                    wt = wpool.tile([P, C, P], BF16, tag="wstat",
                                    name=f"wstk2_{oc}")
                    nc.sync.dma_start(out=wt[:], in_=wstat_ap(kw, oc, C))
                    ps = ps_mm()
                    for kc in range(C):
                        nc.tensor.matmul(ps[:], wt[:, kc, :],
                                         se_own[:, kc, :],
                                         start=(kc == 0), stop=(kc == C - 1))
                    copy_bias(k2[:, oc, 0:R], ps[:], kb_s[:, oc:oc + 1])
                in_b = dramp.tile([2, P, 8 * R], BF16, tag="cc_in")
                red = dramp.tile([2, P, 8 * R], BF16, tag="cc_out")
                nc.gpsimd.dma_start(
                    out=in_b[0].rearrange("p (c r) -> p c r", c=C),
                    in_=k2[:, :, 0:R])

                # V2 row-major [P, rc, D]; own row chunks 0-3
                v2 = p5.tile([P, C, D], BF16, tag="v2")
                for qh in range(2):
                    wm = p5wm.tile([P, C, R], BF16, tag="wmov",
                                   name=f"wmv2_{qh}")
                    nc.sync.dma_start(
                        out=wm[:],
                        in_=vw.ap().rearrange("(k p) m -> p k m", p=P)
                        [:, :, qh * R:(qh + 1) * R])
                    for rc in range(4):
                        ps = ps_mm()
                        for kc in range(C):
                            nc.tensor.matmul(
                                ps[:], se_own[:, kc, rc * P:(rc + 1) * P],
                                wm[:, kc, :], start=(kc == 0),
                                stop=(kc == C - 1 and vb_s is None))
                        if vb_s is not None:
                            nc.tensor.matmul(
                                ps[:], ones_row[:],
                                vb_s[0:1, qh * R:(qh + 1) * R],
                                start=False, stop=True)
                        nc.vector.tensor_scalar(
                            v2[:, rc, qh * R:(qh + 1) * R], ps[:],
                            0.0, None, OP.add)
                nc.gpsimd.dma_start(
                    out=in_b[1].rearrange("p (c r) -> p c r", c=4),
                    in_=v2[:, 0:4, :])
                nc.gpsimd.collective_compute(
                    "AllReduce", OP.add,
                    replica_groups=[[0, 1], [2, 3], [4, 5], [6, 7]],
                    ins=[in_b.opt()], outs=[red.opt()])

                # Q2 projections (independent of the collective)
                q2 = p5.tile([P, C, R], BF16, tag="q2")
                proj(q2, C, se_own, C, qw, qb_s)

                # own-half attention: heads accumulate ctx/denominator over
                # k-chunks 0-3 into SBUF, freeing PSUM before the peer half
                ctx_own = p5.tile([P, C, R], BF16, tag="ctx_own")
                sum_own = p5.tile([1, H_SG, R], F32, tag="sum_own")
                ex_t = {}

                def att2_half(h, rng, ctx_sb, sum_sb, prev_ctx, prev_sum):
                    ctx_ps = [ps_ctx(f"c2ps{mh}") for mh in range(2)]
                    sum_ps = ps_score("a2sum")
                    for j, kc in enumerate(rng):
                        first, last = (j == 0), (j == len(rng) - 1)
                        ps_s = ps_score("score2")
                        for i in range(2):
                            oc = 2 * h + i
                            nc.tensor.matmul(
                                ps_s[:], k2[:, oc, kc * P:(kc + 1) * P],
                                q2[:, oc, :], start=(i == 0), stop=(i == 1))
                        ex = p5ex.tile([P, R], BF16, tag="exp", name="ex2")
                        nc.scalar.activation(ex[:], ps_s[:], AF.Exp,
                                             scale=0.0625)
                        nc.tensor.matmul(sum_ps[0:1, :], ones_col[:], ex[:],
                                         start=first, stop=last)
                        for mh in range(2):
                            nc.tensor.matmul(
                                ctx_ps[mh][:],
                                v2[:, kc, (h * 256 + mh * P):(h * 256 + (mh + 1) * P)],
                                ex[:], start=first, stop=last)
                    if prev_ctx is None:
                        for mh in range(2):
                            nc.vector.tensor_scalar(
                                ctx_sb[:, 2 * h + mh, :], ctx_ps[mh][:],
                                0.0, None, OP.add)
                        nc.vector.tensor_scalar(sum_sb[0:1, h, :],
                                                sum_ps[0:1, :], 0.0, None,
                                                OP.add)
                    else:
                        # combine halves, normalize, write ctx
                        den = small.tile([1, R], F32, tag="a2den", name="den")
                        nc.vector.tensor_add(den[:], sum_ps[0:1, :],
                                             prev_sum[0:1, h, :])
                        rec = small.tile([1, R], BF16, tag="rec", name="rec2")
                        nc.vector.reciprocal(rec[:], den[:])
                        bc_ps = ps_score("bcast2")
                        nc.tensor.matmul(bc_ps[:], ones_row[:], rec[:],
                                         start=True, stop=True)
                        bc_sb = small.tile([P, R], BF16, tag="bcsb",
                                           name="bcsb2")
                        nc.vector.tensor_scalar(bc_sb[:], bc_ps[:], 0.0,
                                                None, OP.add)
                        for mh in range(2):
                            oc = 2 * h + mh
                            tot = small.tile([P, R], BF16, tag="a2tot",
                                             name="tot")
                            nc.vector.tensor_add(tot[:], ctx_ps[mh][:],
                                                 prev_ctx[:, oc, :])
                            nc.vector.tensor_mul(ctx_sb[:, oc, :], tot[:],
                                                 bc_sb[:])

                for h in range(H_SG):
                    att2_half(h, range(4), ctx_own, sum_own, None, None)

                # peer recovery: peer = allreduce_sum - own
                ksum = p5.tile([P, C, R], BF16, tag="ksum")
                nc.sync.dma_start(
                    out=ksum[:],
                    in_=red[0].rearrange("p (c r) -> p c r", c=C))
                for oc in range(C):
                    nc.vector.tensor_sub(k2[:, oc, R:S], ksum[:, oc, :],
                                         k2[:, oc, 0:R])
                vsum = p5.tile([P, 4, D], BF16, tag="vsum")
                nc.sync.dma_start(
                    out=vsum[:],
                    in_=red[1].rearrange("p (c r) -> p c r", c=4))
                for rc in range(4):
                    nc.vector.tensor_sub(v2[:, 4 + rc, :], vsum[:, rc, :],
                                         v2[:, rc, :])

                ctx2 = p5.tile([P, C, R], BF16, tag="ctx2")
                for h in range(H_SG):
                    att2_half(h, range(4, 8), ctx2, None, ctx_own, sum_own)

                # ---------------- phase 6: final residual + LN -------------
                for c in range(C):
                    nc.vector.tensor_add(ctx2[:, c, :], ctx2[:, c, :],
                                         se_own[:, c, :])
                fin = p5.tile([P, C, R], F32, tag="fin")
                layer_norm(ctx2, C, nrg_s, nrb_s, fin)
                nc.sync.dma_start(
                    out=out_d.ap().rearrange("(c p) r -> p c r", p=P),
                    in_=fin[:])
            sep_cm.__exit__(None, None, None)

    nc.compile()
    return nc


def _pc(v):
    """[n*128] -> [128, n] per-partition layout."""
    v = np.asarray(v, np.float32)
    return np.ascontiguousarray(v.reshape(-1, P).T)


def _bf(a):
    return np.ascontiguousarray(np.asarray(a, np.float32)
                                .astype(ml_dtypes.bfloat16))


def kernel(**inputs):
    x = np.asarray(inputs["input_states"], np.float32)
    gate = float(np.asarray(inputs["gate"]).ravel()[0])
    include_cross = (gate != 1.0)

    bq, bk, bv = np.split(np.asarray(inputs["sa_in_b"], np.float32), 3)
    vb = np.asarray(inputs["v_b"], np.float32)
    cbv = (np.split(np.asarray(inputs["ca_in_b"], np.float32), 3)[2]
           if include_cross else np.zeros(1, np.float32))
    with_vbias = bool(np.any(bv) or np.any(vb) or np.any(cbv))

    key = (include_cross, with_vbias)
    if key not in _CACHE:
        _CACHE[key] = _build(include_cross, with_vbias)
    nc = _CACHE[key]

    wq, wk, wv = [_bf(w) for w in
                  np.split(np.asarray(inputs["sa_in_w"], np.float32), 3,
                           axis=1)]

    shared = {
        "wq": wq, "wk": wk, "wv": wv,
        "bq": _pc(bq), "bk": _pc(bk),
        "wo": _bf(inputs["sa_out_w"]),
        "bo": _pc(inputs["sa_out_b"]),
        "w1": _bf(inputs["ffn_w1"]),
        "b1": _pc(inputs["ffn_b1"]),
        "w2": _bf(inputs["ffn_w2"]),
        "b2": _pc(inputs["ffn_b2"]),
        "exw": _bf(inputs["ex_w"]),
        "exb": _pc(inputs["ex_b"]),
        "sqw": _bf(inputs["sq_w"]),
        "sqb": _pc(inputs["sq_b"]),
        "qw": _bf(inputs["q_w"]),
        "qb": _pc(inputs["q_b"]),
        "kw": _bf(inputs["k_w"]),
        "kb": _pc(inputs["k_b"]),
        "vw": _bf(inputs["v_w"]),
        "nsg": _pc(inputs["ns_g"]), "nsb": _pc(inputs["ns_b"]),
        "nfg": _pc(inputs["nf_g"]), "nfb": _pc(inputs["nf_b"]),
        "nrg": _pc(inputs["nrm_g"]), "nrb": _pc(inputs["nrm_b"]),
    }
    if with_vbias:
        shared["bv"] = _bf(bv.reshape(1, D))
        shared["vb"] = _bf(vb.reshape(1, D))
    if include_cross:
        m = np.asarray(inputs["memory_states"], np.float32)
        cwq, cwk, cwv = [_bf(w) for w in
                         np.split(np.asarray(inputs["ca_in_w"], np.float32),
                                  3, axis=1)]
        cbq, cbk, cbv_ = np.split(np.asarray(inputs["ca_in_b"], np.float32), 3)
        shared.update({
            "cwq": cwq, "cwk": cwk, "cwv": cwv,
            "cbq": _pc(cbq), "cbk": _pc(cbk),
            "cwo": _bf(inputs["ca_out_w"]),
            "cbo": _pc(inputs["ca_out_b"]),
            "ncg": _pc(inputs["nc_g"]), "ncb": _pc(inputs["nc_b"]),
            "gate_c": np.full((P, 1), gate, np.float32),
            "gate_1mc": np.full((P, 1), 1.0 - gate, np.float32),
        })
        if with_vbias:
            shared["cbv"] = _bf(cbv_.reshape(1, D))

    in_maps = []
    for c in range(N_CORES):
        b, hf = c // 2, c % 2
        xp = np.concatenate([x[b, hf * R:(hf + 1) * R],
                             x[b, (1 - hf) * R:(2 - hf) * R]], axis=0)
        m_in = dict(shared)
        m_in["xT"] = _bf(xp.T)
        if include_cross:
            m_in["mT"] = _bf(m[b].T)
        in_maps.append(m_in)

    res = bass_utils.run_bass_kernel_spmd(nc, in_maps,
                                          core_ids=list(range(N_CORES)))
    out = np.empty((4, S, D), np.float32)
    for c in range(N_CORES):
        b, hf = c // 2, c % 2
        out[b, hf * R:(hf + 1) * R, :] = res.results[c]["out"].T
    return out


# revision 39
# speedup vs baseline: 1.5207x; 1.2535x over previous
"""Trainium2 Bass kernel for nn_LinearTransformerLayer_44495861187342.

Reference network: SGIRA block (self-attn MHA-16h -> LN -> cross-attn -> LN ->
gate blend -> FFN(gelu) -> LN) followed by a SAIGA block (squeeze-excite MLP ->
LN -> 4-head self-attn -> LN).  With the shipped inputs gate == 1.0, so the
cross-attention branch is algebraically dead and memory_states is unused; a
general path that includes it is kept for gate != 1.

Sharding (8 NeuronCores): core c owns 512 rows = (batch c//2, half c%2) of the
[4, 1024, 1024] input.  Row-local ops (projections, FFN, layernorm, softmax)
shard perfectly.  SGIRA self-attention K/V are recomputed per core for the full
1024-row batch (input replicated host-side).  SAIGA K/V are computed for the
own 512 rows only and exchanged within each core pair via one AllReduce(add);
the peer half is recovered as sum - own, which overlaps the collective with the
own-half attention work.

Layout: activations are feature-major in SBUF ([feat partitions, rows free]) so
matmuls contract the partition dim against natural-layout weights, and
layernorm/softmax feature reductions are PE ones-matmuls.  All matmul operands
are bf16 (fp32 accumulation in PSUM); LN statistics and the final output are
fp32.  Scalar engine handles exp/gelu; all other PSUM->SBUF copies run on the
vector engine with the bias folded in.
"""

import contextlib

import ml_dtypes
import numpy as np

import concourse.bass as bass
import concourse.mybir as mybir
import concourse.tile as tile
from concourse import bacc
from concourse import bass_utils

F32 = mybir.dt.float32
F32R = mybir.dt.float32r
BF16 = mybir.dt.bfloat16
AF = mybir.ActivationFunctionType
OP = mybir.AluOpType

D = 1024          # model dim
DFF = 4096        # ffn dim
D2 = 2048         # squeeze-excite dim
S = 1024          # full sequence rows per batch
R = 512           # rows owned per core
P = 128
C = D // P        # 8 feature chunks
CFF = DFF // P    # 32
C2 = D2 // P      # 16
H_SA = 16         # SGIRA heads (hd 64)
H_SG = 4          # SAIGA heads (hd 256)
N_CORES = 8
EPS = 1e-5

_CACHE = {}
_DBG_TAPS = False  # set by dbg script only: dumps ss/hidden/se intermediates


def _build(include_cross: bool, with_vbias: bool):
    nc = bacc.Bacc("TRN2", target_bir_lowering=False, debug=False,
                   num_devices=N_CORES)

    def din(name, shape, dt=BF16):
        return nc.dram_tensor(name, shape, dt, kind="ExternalInput")

    # feature-major inputs (host pre-transposed), own 512 rows first
    xT = din("xT", [D, S])
    wq = din("wq", [D, D]); wk = din("wk", [D, D]); wv = din("wv", [D, D])
    bq = din("bq", [P, C], F32); bk = din("bk", [P, C], F32)
    wo = din("wo", [D, D]); bo = din("bo", [P, C], F32)
    w1 = din("w1", [D, DFF]); b1 = din("b1", [P, CFF], F32)
    w2 = din("w2", [DFF, D]); b2 = din("b2", [P, C], F32)
    exw = din("exw", [D, D2]); exb = din("exb", [P, C2], F32)
    sqw = din("sqw", [D2, D]); sqb = din("sqb", [P, C], F32)
    qw = din("qw", [D, D]); qb = din("qb", [P, C], F32)
    kw = din("kw", [D, D]); kb = din("kb", [P, C], F32)
    vw = din("vw", [D, D])
    nsg = din("nsg", [P, C], F32); nsb = din("nsb", [P, C], F32)
    nfg = din("nfg", [P, C], F32); nfb = din("nfb", [P, C], F32)
    nrg = din("nrg", [P, C], F32); nrb = din("nrb", [P, C], F32)
    if with_vbias:
        bv = din("bv", [1, D])
        vb = din("vb", [1, D])
    if include_cross:
        mT = din("mT", [D, S])
        cwq = din("cwq", [D, D]); cwk = din("cwk", [D, D]); cwv = din("cwv", [D, D])
        cbq = din("cbq", [P, C], F32); cbk = din("cbk", [P, C], F32)
        cwo = din("cwo", [D, D]); cbo = din("cbo", [P, C], F32)
        ncg = din("ncg", [P, C], F32); ncb = din("ncb", [P, C], F32)
        gate_c = din("gate_c", [P, 1], F32)      # broadcast gate
        gate_1mc = din("gate_1mc", [P, 1], F32)  # broadcast (1 - gate)
        if with_vbias:
            cbv = din("cbv", [1, D])

    out_d = nc.dram_tensor("out", [D, R], F32, kind="ExternalOutput")
    if _DBG_TAPS:
        tap_d = {nm: nc.dram_tensor(f"tap_{nm}", [D, R], F32,
                                    kind="ExternalOutput")
                 for nm in ("ctxsa", "ss", "hidden", "se", "ctx2")}

    with tile.TileContext(nc) as tc:
        with contextlib.ExitStack() as ctx, \
             nc.allow_low_precision("bf16 operands feeding the PE"):
            const = ctx.enter_context(tc.tile_pool(name="const", bufs=1))
            wpool = ctx.enter_context(tc.tile_pool(name="wpool", bufs=4))
            # PSUM: mm 2 + score 4 + ctx 2 = 8 banks
            psp = ctx.enter_context(tc.tile_pool(name="psp", bufs=1,
                                                 space="PSUM"))
            small = ctx.enter_context(tc.tile_pool(name="small", bufs=2))

            # PSUM budget: mm 2x[P,R] + score 2x[P,2R] + ctx 2x[P,R] = 8 banks
            def ps_mm(name="psmm"):
                return psp.tile([P, R], F32, tag="mm", bufs=2, name=name)

            def ps_score(name="score"):
                return psp.tile([P, R], F32, tag="score", bufs=2, name=name)

            def ps_score2(name="score2"):
                return psp.tile([P, 2 * R], F32, tag="score", bufs=2,
                                name=name)

            def ps_ctx(name="ctxps"):
                return psp.tile([P, R], F32, tag="ctx", bufs=2, name=name)

            ones_col = const.tile([P, 1], BF16, tag="ones_col")
            nc.vector.memset(ones_col[:], 1.0)
            ones_row = const.tile([1, P], BF16, tag="ones_row")
            nc.vector.memset(ones_row[:], 1.0)
            ones_f = const.tile([1, P], F32, tag="ones_f")
            nc.vector.memset(ones_f[:], 1.0)
            ones_row_r = const.tile([1, P], F32R, tag="ones_row_r")
            nc.scalar.copy(ones_row_r[:], ones_f[:])
            eps_t = const.tile([1, 1], F32, tag="eps")
            nc.vector.memset(eps_t[:], EPS)

            from concourse.dve_ops import (
                RECIP_APPROX_FAST_CONSTS,
                RECIPROCAL_APPROX_FAST,
            )

            def recip_r(out_r, in_ap):
                """~18-bit 1/x straight into an f32r-typed tile (single DVE
                op; the f32r output dtype satisfies the BIR verifier for
                downstream f32r matmuls)."""
                c = RECIP_APPROX_FAST_CONSTS
                nc.vector._custom_dve(
                    RECIPROCAL_APPROX_FAST, out=out_r, in0=in_ap,
                    s0=c["s0"], s1=c["s1"], imm2=c["imm2"])

            # HAM warmup: ~5us of dummy matmuls so the PE clock is at 2.4GHz
            # by the time the first real matmul's inputs arrive from HBM.
            warm = const.tile([P, R], BF16, tag="warm")
            nc.vector.memset(warm[:], 0.001)
            wu_ps = psp.tile([P, R], F32, tag="ctx", bufs=2, name="warmups")
            for i in range(12):
                nc.tensor.matmul(wu_ps[:], warm[:, 0:P], warm[:],
                                 start=(i == 0), stop=(i == 11))
            wu_sb = const.tile([1, 1], F32, tag="warmsb")
            nc.vector.tensor_scalar(wu_sb[:], wu_ps[0:1, 0:1], 0.0, None,
                                    OP.add)

            def bcast_rows(rec_r):
                """[1, R] f32r -> [P, R] psum broadcast via an f32r matmul."""
                bc_ps = ps_score("bcast")
                nc.tensor.matmul(bc_ps[:], ones_row_r[:], rec_r[:],
                                 start=True, stop=True)
                return bc_ps

            def load_pc(dram):  # small per-partition tables
                t = const.tile(list(dram.shape), dram.dtype,
                               tag=dram.name + "_sb")
                nc.sync.dma_start(out=t[:], in_=dram.ap())
                return t

            def wstat_ap(w, oc, kcs):
                # [P, kcs, 128] stationary block: w[(kc p), oc*128 + m]
                return w.ap().rearrange("(k p) m -> p k m", p=P)[:, :, oc * P:(oc + 1) * P]

            def tap(nm, t):
                if _DBG_TAPS:
                    f32t = const.tile([P, C, R], F32, tag="tapbuf")
                    for c in range(C):
                        nc.vector.tensor_scalar(f32t[:, c, :], t[:, c, :],
                                                0.0, None, OP.add)
                    nc.sync.dma_start(
                        out=tap_d[nm].ap().rearrange("(c p) r -> p c r", p=P),
                        in_=f32t[:])

            def copy_bias(out_ap, ps, bias_s, idx=0):
                """PSUM -> SBUF copy with per-partition bias; alternates
                between DVE and ACT so neither engine gates the PE."""
                if idx % 2 == 0:
                    nc.vector.tensor_scalar(out_ap, ps, bias_s, None, OP.add)
                else:
                    nc.scalar.activation(out_ap, ps, AF.Identity, bias=bias_s)

            def proj(out_t, out_c, in_t, in_c, w, bias_s, func=None):
                """out_t[:, oc, r] = func(sum_k w[k, oc*128+p] * in_t[k, r] + b)

                over the own R rows of in_t."""
                for oc in range(out_c):
                    wt = wpool.tile([P, in_c, P], BF16, tag="wstat",
                                    name=f"wst_{w.name}{oc}")
                    nc.sync.dma_start(out=wt[:], in_=wstat_ap(w, oc, in_c))
                    ps = ps_mm()
                    for kc in range(in_c):
                        nc.tensor.matmul(ps[:], wt[:, kc, :], in_t[:, kc, 0:R],
                                         start=(kc == 0), stop=(kc == in_c - 1))
                    if func == "gelu":
                        nc.scalar.activation(out_t[:, oc, :], ps[:], AF.Gelu,
                                             bias=bias_s[:, oc:oc + 1])
                    elif func == "relu":
                        nc.vector.tensor_scalar(out_t[:, oc, :], ps[:],
                                                bias_s[:, oc:oc + 1], 0.0,
                                                OP.add, OP.max)
                    else:
                        copy_bias(out_t[:, oc, :], ps[:], bias_s[:, oc:oc + 1],
                                  oc)

            def layer_norm(a_t, n_c, g_s, b_s, out_t):
                """Row-wise LN over the (P * n_c) feature dim of a_t [P,n_c,R]."""
                inv_n = 1.0 / (n_c * P)
                ps_sum = ps_score("lnsum")
                ps_sq = ps_score("lnsumsq")
                for c in range(n_c):
                    sqc = small.tile([P, R], BF16, tag="lnsqc", name="lnsqc")
                    nc.vector.tensor_mul(sqc[:], a_t[:, c, :], a_t[:, c, :])
                    nc.tensor.matmul(ps_sum[0:1, :], ones_col[:], a_t[:, c, :],
                                     start=(c == 0), stop=(c == n_c - 1))
                    nc.tensor.matmul(ps_sq[0:1, :], ones_col[:], sqc[:],
                                     start=(c == 0), stop=(c == n_c - 1))
                mu = small.tile([1, R], F32R, tag="lnmu", name="lnmu")
                nc.scalar.activation(mu[:], ps_sum[0:1, :], AF.Copy,
                                     scale=inv_n)
                va = small.tile([1, R], F32, tag="lnva", name="lnva")
                nc.scalar.activation(va[:], ps_sq[0:1, :], AF.Copy,
                                     scale=inv_n)
                mu2 = small.tile([1, R], F32, tag="lnmu2", name="lnmu2")
                nc.vector.tensor_mul(mu2[:], mu[:].bitcast(F32),
                                     mu[:].bitcast(F32))
                nc.vector.tensor_sub(va[:], va[:], mu2[:])
                nc.scalar.activation(va[:], va[:], AF.Sqrt, bias=eps_t[:])
                rstd = small.tile([1, R], F32R, tag="lnrstd", name="lnrstd")
                recip_r(rstd[:], va[:])
                # out = ((x - mu_bc) * rstd_bc) * g + b
                bcb_ps = bcast_rows(mu)     # broadcast mu
                bca_ps = bcast_rows(rstd)   # broadcast rstd
                bcb = small.tile([P, R], BF16, tag="lnbcbs", name="lnbcbs")
                nc.scalar.activation(bcb[:], bcb_ps[:], AF.Copy)
                bca = small.tile([P, R], BF16, tag="lnbcas", name="lnbcas")
                nc.vector.tensor_scalar(bca[:], bca_ps[:], 0.0, None, OP.add)
                for c in range(n_c):
                    nc.vector.tensor_sub(out_t[:, c, :], a_t[:, c, :], bcb[:])
                    nc.vector.tensor_mul(out_t[:, c, :], out_t[:, c, :],
                                         bca[:])
                    nc.vector.tensor_scalar(
                        out_t[:, c, :], out_t[:, c, :],
                        g_s[:, c:c + 1], b_s[:, c:c + 1], OP.mult, OP.add)

            def vproj_16(kv_src, w_v, vbias_t, vpool, wmpool, rcs, tag):
                """Row-major V for 16-head attention: [P, rc, 16, 65] with a
                ones column per head (softmax denominator trick)."""
                v4 = vpool.tile([P, len(rcs), H_SA, 65], BF16, tag=tag)
                nc.vector.memset(v4[:, :, :, 64:65], 1.0)
                for qh in range(2):
                    wm = wmpool.tile([P, C, R], BF16, tag="wmov",
                                     name=f"wm_{tag}{qh}")
                    nc.sync.dma_start(
                        out=wm[:],
                        in_=w_v.ap().rearrange("(k p) m -> p k m", p=P)
                        [:, :, qh * R:(qh + 1) * R])
                    for i, rc in enumerate(rcs):
                        ps = ps_mm()
                        for kc in range(C):
                            nc.tensor.matmul(
                                ps[:], kv_src[:, kc, rc * P:(rc + 1) * P],
                                wm[:, kc, :], start=(kc == 0),
                                stop=(kc == C - 1 and vbias_t is None))
                        if vbias_t is not None:
                            nc.tensor.matmul(
                                ps[:], ones_row[:],
                                vbias_t[0:1, qh * R:(qh + 1) * R],
                                start=False, stop=True)
                        if rc % 2 == 0:
                            nc.vector.tensor_scalar(
                                v4[:, i, qh * 8:(qh + 1) * 8, 0:64],
                                ps.rearrange("p (h e) -> p h e", e=64),
                                0.0, None, OP.add)
                        else:
                            nc.scalar.activation(
                                v4[:, i, qh * 8:(qh + 1) * 8, 0:64],
                                ps.rearrange("p (h e) -> p h e", e=64),
                                AF.Copy)
                return v4

            def mha16(kv_src, q_src, w_q, w_k, w_v, bq_t, bk_t, bv_t, pools):
                """16-head attention; q over own R rows, k/v over S rows of
                kv_src.  Returns ctx feature-major [P, C, R]."""
                apool, vpool, kqpool, expool, wmpool = pools
                v4 = vproj_16(kv_src, w_v, bv_t, vpool, wmpool,
                              list(range(C)), "v_sa")
                ctx_t = apool.tile([P, C, R], BF16, tag="ctx_sa",
                                   name="ctx_sa")
                kq = {}

                def kq_proj(oc):
                    wt = wpool.tile([P, C, P], BF16, tag="wstat",
                                    name=f"wstk{oc}")
                    nc.sync.dma_start(out=wt[:], in_=wstat_ap(w_k, oc, C))
                    kf = kqpool.tile([P, S], BF16, tag="kf", name="kf")
                    for rh in range(2):
                        ps = ps_mm()
                        for kc in range(C):
                            nc.tensor.matmul(
                                ps[:], wt[:, kc, :],
                                kv_src[:, kc, rh * R:(rh + 1) * R],
                                start=(kc == 0), stop=(kc == C - 1))
                        copy_bias(kf[:, rh * R:(rh + 1) * R], ps[:],
                                  bk_t[:, oc:oc + 1], rh)
                    wtq = wpool.tile([P, C, P], BF16, tag="wstat",
                                     name=f"wstq{oc}")
                    nc.sync.dma_start(out=wtq[:], in_=wstat_ap(w_q, oc, C))
                    qf = kqpool.tile([P, R], BF16, tag="qf", name="qf")
                    ps = ps_mm()
                    for kc in range(C):
                        nc.tensor.matmul(ps[:], wtq[:, kc, :],
                                         q_src[:, kc, 0:R],
                                         start=(kc == 0), stop=(kc == C - 1))
                    copy_bias(qf[:], ps[:], bq_t[:, oc:oc + 1], oc)
                    kq[oc] = (kf, qf)

                def att(oc):
                    kf, qf = kq.pop(oc)
                    ctx_ps = [ps_ctx(f"ctxps{hh}") for hh in range(2)]
                    for kc in range(C):
                        # row-tiled concurrent score pair (heads 2oc, 2oc+1)
                        # into the two banks of one [P, 2R] tile, then one
                        # batched exp for both heads
                        ps_s = ps_score2("scoreAB")
                        for hh in range(2):
                            po = hh * 64
                            nc.tensor.matmul(
                                ps_s[:, hh * R:(hh + 1) * R],
                                kf[po:po + 64, kc * P:(kc + 1) * P],
                                qf[po:po + 64, :], start=True, stop=True)
                        ex = expool.tile([P, 2 * R], BF16, tag="exp",
                                         name="exAB")
                        nc.scalar.activation(ex[:], ps_s[:], AF.Exp,
                                             scale=0.125)
                        for hh in range(2):
                            h = oc * 2 + hh
                            nc.tensor.matmul(
                                ctx_ps[hh][:65, :],
                                v4[:, kc, h, 0:65],
                                ex[:, hh * R:(hh + 1) * R],
                                start=(kc == 0), stop=(kc == C - 1))
                    for hh in range(2):
                        po = hh * 64
                        # custom-DVE ops misread PSUM at partition offsets;
                        # stage the denominator row through SBUF first
                        den = small.tile([1, R], F32, tag="attden",
                                         name="attden")
                        nc.scalar.activation(den[:], ctx_ps[hh][64:65, :],
                                             AF.Copy)
                        rec = small.tile([1, R], F32R, tag="rec", name="rec")
                        recip_r(rec[:], den[:])
                        bc_ps = ps_score("bcast")
                        nc.tensor.matmul(bc_ps[0:64, :],
                                         ones_row_r[0:1, 0:64], rec[:],
                                         start=True, stop=True)
                        bc_sb = small.tile([64, R], BF16, tag="bcsb",
                                           name="bcsb")
                        nc.scalar.activation(bc_sb[:], bc_ps[0:64, :],
                                             AF.Copy)
                        nc.vector.tensor_mul(ctx_t[po:po + 64, oc, :],
                                             ctx_ps[hh][0:64, :], bc_sb[:])

                kq_proj(0)
                kq_proj(1)
                for oc in range(C):
                    if oc + 2 < C:
                        kq_proj(oc + 2)
                    att(oc)
                return ctx_t

            # ---------------- phase 1: SGIRA self-attention ----------------
            ssp_cm = tc.tile_pool(name="ssp", bufs=1, side="right")
            ssp = ssp_cm.__enter__()
            ss = ssp.tile([P, C, R], BF16, tag="ss")
            with tc.tile_pool(name="p1", bufs=1) as p1, \
                 tc.tile_pool(name="p1kq", bufs=2) as p1kq, \
                 tc.tile_pool(name="p1ex", bufs=3) as p1ex, \
                 tc.tile_pool(name="p1wm", bufs=2) as p1wm:
                xT_s = p1.tile([P, C, S], BF16, tag="xT")
                xt_ap = xT.ap().rearrange("(c p) r -> p c r", p=P)
                for c in range(C):
                    nc.sync.dma_start(out=xT_s[:, c, :], in_=xt_ap[:, c, :])
                bq_s = load_pc(bq); bk_s = load_pc(bk)
                bv_s = load_pc(bv) if with_vbias else None
                ctx_sa = mha16(xT_s, xT_s, wq, wk, wv, bq_s, bk_s, bv_s,
                               (p1, p1, p1kq, p1ex, p1wm))
                # out-proj + residual + LN -> ss
                bo_s = load_pc(bo)
                nsg_s = load_pc(nsg); nsb_s = load_pc(nsb)
                tap("ctxsa", ctx_sa)
                sa = p1.tile([P, C, R], BF16, tag="sa")
                proj(sa, C, ctx_sa, C, wo, bo_s)
                for c in range(C):
                    nc.vector.tensor_add(sa[:, c, :], sa[:, c, :],
                                         xT_s[:, c, 0:R])
                layer_norm(sa, C, nsg_s, nsb_s, ss)
                tap("ss", ss)

            # ---------------- optional cross-attention (gate != 1) ---------
            if include_cross:
                fusedp_cm = tc.tile_pool(name="fusedp", bufs=1, side="right")
                fusedp = fusedp_cm.__enter__()
                fused = fusedp.tile([P, C, R], BF16, tag="fused")
                with tc.tile_pool(name="pc1", bufs=1) as pc1, \
                     tc.tile_pool(name="pc1kq", bufs=2) as pc1kq, \
                     tc.tile_pool(name="pc1ex", bufs=3) as pc1ex, \
                     tc.tile_pool(name="pc1wm", bufs=2) as pc1wm:
                    mT_s = pc1.tile([P, C, S], BF16, tag="mT")
                    mt_ap = mT.ap().rearrange("(c p) r -> p c r", p=P)
                    for c in range(C):
                        nc.sync.dma_start(out=mT_s[:, c, :], in_=mt_ap[:, c, :])
                    cbq_s = load_pc(cbq); cbk_s = load_pc(cbk)
                    cbv_s = load_pc(cbv) if with_vbias else None
                    ctx_ca = mha16(mT_s, ss, cwq, cwk, cwv, cbq_s, cbk_s,
                                   cbv_s, (pc1, pc1, pc1kq, pc1ex, pc1wm))
                    cbo_s = load_pc(cbo)
                    ncg_s = load_pc(ncg); ncb_s = load_pc(ncb)
                    gc_s = load_pc(gate_c); g1_s = load_pc(gate_1mc)
                    ca = pc1.tile([P, C, R], BF16, tag="ca")
                    proj(ca, C, ctx_ca, C, cwo, cbo_s)
                    for c in range(C):
                        nc.vector.tensor_add(ca[:, c, :], ca[:, c, :],
                                             ss[:, c, :])
                    cs = pc1.tile([P, C, R], BF16, tag="cs")
                    layer_norm(ca, C, ncg_s, ncb_s, cs)
                    # fused = gate*ss + (1-gate)*cs
                    for c in range(C):
                        nc.vector.tensor_scalar(
                            fused[:, c, :], ss[:, c, :], gc_s[:, 0:1], None,
                            OP.mult)
                        nc.vector.tensor_scalar(
                            cs[:, c, :], cs[:, c, :], g1_s[:, 0:1], None,
                            OP.mult)
                        nc.vector.tensor_add(fused[:, c, :], fused[:, c, :],
                                             cs[:, c, :])
                ff_in = fused
            else:
                ff_in = ss

            # ---------------- phase 2: FFN ----------------
            hidp_cm = tc.tile_pool(name="hidp", bufs=1)
            hidp = hidp_cm.__enter__()
            hidden = hidp.tile([P, C, R], BF16, tag="hidden")
            with tc.tile_pool(name="p2", bufs=1) as p2:
                b1_s = load_pc(b1); b2_s = load_pc(b2)
                nfg_s = load_pc(nfg); nfb_s = load_pc(nfb)
                h1 = p2.tile([P, CFF, R], BF16, tag="h1")
                proj(h1, CFF, ff_in, C, w1, b1_s, func="gelu")
                ffo = p2.tile([P, C, R], BF16, tag="ffo")
                for oc in range(C):
                    wt2 = p2.tile([P, CFF, P], BF16, tag="wstat2", bufs=3,
                                  name=f"wst2_{oc}")
                    nc.sync.dma_start(out=wt2[:], in_=wstat_ap(w2, oc, CFF))
                    ps = ps_mm()
                    for kc in range(CFF):
                        nc.tensor.matmul(ps[:], wt2[:, kc, :], h1[:, kc, :],
                                         start=(kc == 0), stop=(kc == CFF - 1))
                    copy_bias(ffo[:, oc, :], ps[:], b2_s[:, oc:oc + 1])
                for c in range(C):
                    nc.vector.tensor_add(ffo[:, c, :], ffo[:, c, :],
                                         ff_in[:, c, :])
                layer_norm(ffo, C, nfg_s, nfb_s, hidden)
                tap("hidden", hidden)
            # ss (or fused) no longer needed
            if include_cross:
                fusedp_cm.__exit__(None, None, None)
            ssp_cm.__exit__(None, None, None)

            # ---------------- phase 3: SAIGA squeeze-excite ----------------
            sep_cm = tc.tile_pool(name="sep", bufs=1, side="right")
            sep = sep_cm.__enter__()
            se_own = sep.tile([P, C, R], BF16, tag="se_own")
            with tc.tile_pool(name="p3", bufs=1) as p3:
                exb_s = load_pc(exb); sqb_s = load_pc(sqb)
                nrg_s = load_pc(nrg); nrb_s = load_pc(nrb)
                h2 = p3.tile([P, C2, R], BF16, tag="h2")
                proj(h2, C2, hidden, C, exw, exb_s, func="relu")
                sqo = p3.tile([P, C, R], BF16, tag="sqo")
                proj(sqo, C, h2, C2, sqw, sqb_s)
                for c in range(C):
                    nc.vector.tensor_add(sqo[:, c, :], sqo[:, c, :],
                                         hidden[:, c, :])
                layer_norm(sqo, C, nrg_s, nrb_s, se_own)
                tap("se", se_own)
            hidp_cm.__exit__(None, None, None)

            # ------- phase 4: SAIGA K/V own-half + pair AllReduce ----------
            # ------- phase 5: 4-head attention (own half overlaps the cc) --
            with tc.tile_pool(name="p5", bufs=1) as p5, \
                 tc.tile_pool(name="p5ex", bufs=3) as p5ex, \
                 tc.tile_pool(name="p5wm", bufs=2) as p5wm, \
                 tc.tile_pool(name="dramp", bufs=1, space="DRAM") as dramp:
                qb_s = load_pc(qb); kb_s = load_pc(kb)
                vb_s = load_pc(vb) if with_vbias else None

                # K2 feature-major [P, C, S]; own rows first
                k2 = p5.tile([P, C, S], BF16, tag="k2")
                for oc in range(C):
                    wt = wpool.tile([P, C, P], BF16, tag="wstat",
                                    name=f"wstk2_{oc}")
                    nc.sync.dma_start(out=wt[:], in_=wstat_ap(kw, oc, C))
                    ps = ps_mm()
                    for kc in range(C):
                        nc.tensor.matmul(ps[:], wt[:, kc, :],
                                         se_own[:, kc, :],
                                         start=(kc == 0), stop=(kc == C - 1))
                    copy_bias(k2[:, oc, 0:R], ps[:], kb_s[:, oc:oc + 1], oc)
                in_b = dramp.tile([2, P, 8 * R], BF16, tag="cc_in")
                red = dramp.tile([2, P, 8 * R], BF16, tag="cc_out")
                nc.gpsimd.dma_start(
                    out=in_b[0].rearrange("p (c r) -> p c r", c=C),
                    in_=k2[:, :, 0:R])

                # V2 row-major [P, rc, D]; own row chunks 0-3
                v2 = p5.tile([P, C, D], BF16, tag="v2")
                for qh in range(2):
                    wm = p5wm.tile([P, C, R], BF16, tag="wmov",
                                   name=f"wmv2_{qh}")
                    nc.sync.dma_start(
                        out=wm[:],
                        in_=vw.ap().rearrange("(k p) m -> p k m", p=P)
                        [:, :, qh * R:(qh + 1) * R])
                    for rc in range(4):
                        ps = ps_mm()
                        for kc in range(C):
                            nc.tensor.matmul(
                                ps[:], se_own[:, kc, rc * P:(rc + 1) * P],
                                wm[:, kc, :], start=(kc == 0),
                                stop=(kc == C - 1 and vb_s is None))
                        if vb_s is not None:
                            nc.tensor.matmul(
                                ps[:], ones_row[:],
                                vb_s[0:1, qh * R:(qh + 1) * R],
                                start=False, stop=True)
                        if rc % 2 == 0:
                            nc.vector.tensor_scalar(
                                v2[:, rc, qh * R:(qh + 1) * R], ps[:],
                                0.0, None, OP.add)
                        else:
                            nc.scalar.activation(
                                v2[:, rc, qh * R:(qh + 1) * R], ps[:],
                                AF.Copy)
                    nc.gpsimd.dma_start(
                        out=in_b[1].rearrange("p (c r) -> p c r", c=4)
                        [:, :, qh * R:(qh + 1) * R],
                        in_=v2[:, 0:4, qh * R:(qh + 1) * R])
                nc.gpsimd.collective_compute(
                    "AllReduce", OP.add,
                    replica_groups=[[0, 1], [2, 3], [4, 5], [6, 7]],
                    ins=[in_b.opt()], outs=[red.opt()])

                # Q2 projections (independent of the collective)
                q2 = p5.tile([P, C, R], BF16, tag="q2")
                proj(q2, C, se_own, C, qw, qb_s)

                # own-half attention: heads accumulate ctx/denominator over
                # k-chunks 0-3 into SBUF, freeing PSUM before the peer half
                ctx_own = p5.tile([P, C, R], BF16, tag="ctx_own")
                sum_own = p5.tile([1, H_SG, R], F32, tag="sum_own")
                ex_t = {}

                def att2_half(h, rng, ctx_sb, sum_sb, prev_ctx, prev_sum):
                    ctx_ps = [ps_ctx(f"c2ps{mh}") for mh in range(2)]
                    sum_ps = ps_mm("a2sum")
                    for j, kc in enumerate(rng):
                        first, last = (j == 0), (j == len(rng) - 1)
                        ps_s = ps_score("score2")
                        for i in range(2):
                            oc = 2 * h + i
                            nc.tensor.matmul(
                                ps_s[:], k2[:, oc, kc * P:(kc + 1) * P],
                                q2[:, oc, :], start=(i == 0), stop=(i == 1))
                        ex = p5ex.tile([P, R], BF16, tag="exp", name="ex2")
                        nc.scalar.activation(ex[:], ps_s[:], AF.Exp,
                                             scale=0.0625)
                        nc.tensor.matmul(sum_ps[0:1, :], ones_col[:], ex[:],
                                         start=first, stop=last)
                        for mh in range(2):
                            nc.tensor.matmul(
                                ctx_ps[mh][:],
                                v2[:, kc, (h * 256 + mh * P):(h * 256 + (mh + 1) * P)],
                                ex[:], start=first, stop=last)
                    if prev_ctx is None:
                        nc.vector.tensor_scalar(
                            ctx_sb[:, 2 * h, :], ctx_ps[0][:],
                            0.0, None, OP.add)
                        nc.scalar.activation(ctx_sb[:, 2 * h + 1, :],
                                             ctx_ps[1][:], AF.Copy)
                        nc.vector.tensor_scalar(sum_sb[0:1, h, :],
                                                sum_ps[0:1, :], 0.0, None,
                                                OP.add)
                    else:
                        # combine halves, normalize, write ctx
                        den = small.tile([1, R], F32, tag="a2den", name="den")
                        nc.vector.tensor_add(den[:], sum_ps[0:1, :],
                                             prev_sum[0:1, h, :])
                        rec = small.tile([1, R], F32R, tag="rec", name="rec2")
                        recip_r(rec[:], den[:])
                        bc_ps = bcast_rows(rec)
                        bc_sb = small.tile([P, R], BF16, tag="bcsb",
                                           name="bcsb2")
                        nc.scalar.activation(bc_sb[:], bc_ps[:], AF.Copy)
                        for mh in range(2):
                            oc = 2 * h + mh
                            tot = small.tile([P, R], BF16, tag="a2tot",
                                             name="tot")
                            nc.vector.tensor_add(tot[:], ctx_ps[mh][:],
                                                 prev_ctx[:, oc, :])
                            nc.vector.tensor_mul(ctx_sb[:, oc, :], tot[:],
                                                 bc_sb[:])

                for h in range(H_SG):
                    att2_half(h, range(4), ctx_own, sum_own, None, None)

                # peer recovery: peer = allreduce_sum - own
                ksum = p5.tile([P, C, R], BF16, tag="ksum")
                nc.sync.dma_start(
                    out=ksum[:],
                    in_=red[0].rearrange("p (c r) -> p c r", c=C))
                for oc in range(C):
                    nc.vector.tensor_sub(k2[:, oc, R:S], ksum[:, oc, :],
                                         k2[:, oc, 0:R])
                vsum = p5.tile([P, 4, D], BF16, tag="vsum")
                nc.sync.dma_start(
                    out=vsum[:],
                    in_=red[1].rearrange("p (c r) -> p c r", c=4))
                for rc in range(4):
                    nc.vector.tensor_sub(v2[:, 4 + rc, :], vsum[:, rc, :],
                                         v2[:, rc, :])

                ctx2 = p5.tile([P, C, R], BF16, tag="ctx2")
                for h in range(H_SG):
                    att2_half(h, range(4, 8), ctx2, None, ctx_own, sum_own)

                tap("ctx2", ctx2)
                # ---------------- phase 6: final residual + LN -------------
                for c in range(C):
                    nc.vector.tensor_add(ctx2[:, c, :], ctx2[:, c, :],
                                         se_own[:, c, :])
                fin = p5.tile([P, C, R], F32, tag="fin")
                layer_norm(ctx2, C, nrg_s, nrb_s, fin)
                nc.sync.dma_start(
                    out=out_d.ap().rearrange("(c p) r -> p c r", p=P),
                    in_=fin[:])
            sep_cm.__exit__(None, None, None)

    nc.compile()
    return nc


def _pc(v):
    """[n*128] -> [128, n] per-partition layout."""
    v = np.asarray(v, np.float32)
    return np.ascontiguousarray(v.reshape(-1, P).T)


def _bf(a):
    return np.ascontiguousarray(np.asarray(a, np.float32)
                                .astype(ml_dtypes.bfloat16))


def kernel(**inputs):
    x = np.asarray(inputs["input_states"], np.float32)
    gate = float(np.asarray(inputs["gate"]).ravel()[0])
    include_cross = (gate != 1.0)

    bq, bk, bv = np.split(np.asarray(inputs["sa_in_b"], np.float32), 3)
    vb = np.asarray(inputs["v_b"], np.float32)
    cbv = (np.split(np.asarray(inputs["ca_in_b"], np.float32), 3)[2]
           if include_cross else np.zeros(1, np.float32))
    with_vbias = bool(np.any(bv) or np.any(vb) or np.any(cbv))

    key = (include_cross, with_vbias)
    if key not in _CACHE:
        _CACHE[key] = _build(include_cross, with_vbias)
    nc = _CACHE[key]

    wq, wk, wv = [_bf(w) for w in
                  np.split(np.asarray(inputs["sa_in_w"], np.float32), 3,
                           axis=1)]

    shared = {
        "wq": wq, "wk": wk, "wv": wv,
        "bq": _pc(bq), "bk": _pc(bk),
        "wo": _bf(inputs["sa_out_w"]),
        "bo": _pc(inputs["sa_out_b"]),
        "w1": _bf(inputs["ffn_w1"]),
        "b1": _pc(inputs["ffn_b1"]),
        "w2": _bf(inputs["ffn_w2"]),
        "b2": _pc(inputs["ffn_b2"]),
        "exw": _bf(inputs["ex_w"]),
        "exb": _pc(inputs["ex_b"]),
        "sqw": _bf(inputs["sq_w"]),
        "sqb": _pc(inputs["sq_b"]),
        "qw": _bf(inputs["q_w"]),
        "qb": _pc(inputs["q_b"]),
        "kw": _bf(inputs["k_w"]),
        "kb": _pc(inputs["k_b"]),
        "vw": _bf(inputs["v_w"]),
        "nsg": _pc(inputs["ns_g"]), "nsb": _pc(inputs["ns_b"]),
        "nfg": _pc(inputs["nf_g"]), "nfb": _pc(inputs["nf_b"]),
        "nrg": _pc(inputs["nrm_g"]), "nrb": _pc(inputs["nrm_b"]),
    }
    if with_vbias:
        shared["bv"] = _bf(bv.reshape(1, D))
        shared["vb"] = _bf(vb.reshape(1, D))
    if include_cross:
        m = np.asarray(inputs["memory_states"], np.float32)
        cwq, cwk, cwv = [_bf(w) for w in
                         np.split(np.asarray(inputs["ca_in_w"], np.float32),
                                  3, axis=1)]
        cbq, cbk, cbv_ = np.split(np.asarray(inputs["ca_in_b"], np.float32), 3)
        shared.update({
            "cwq": cwq, "cwk": cwk, "cwv": cwv,
            "cbq": _pc(cbq), "cbk": _pc(cbk),
            "cwo": _bf(inputs["ca_out_w"]),
            "cbo": _pc(inputs["ca_out_b"]),
            "ncg": _pc(inputs["nc_g"]), "ncb": _pc(inputs["nc_b"]),
            "gate_c": np.full((P, 1), gate, np.float32),
            "gate_1mc": np.full((P, 1), 1.0 - gate, np.float32),
        })
        if with_vbias:
            shared["cbv"] = _bf(cbv_.reshape(1, D))

    in_maps = []
    for c in range(N_CORES):
        b, hf = c // 2, c % 2
        xp = np.concatenate([x[b, hf * R:(hf + 1) * R],
                             x[b, (1 - hf) * R:(2 - hf) * R]], axis=0)
        m_in = dict(shared)
        m_in["xT"] = _bf(xp.T)
        if include_cross:
            m_in["mT"] = _bf(m[b].T)
        in_maps.append(m_in)

    res = bass_utils.run_bass_kernel_spmd(nc, in_maps,
                                          core_ids=list(range(N_CORES)))
    out = np.empty((4, S, D), np.float32)
    for c in range(N_CORES):
        b, hf = c // 2, c % 2
        out[b, hf * R:(hf + 1) * R, :] = res.results[c]["out"].T
    return out


# revision 53
# speedup vs baseline: 1.5738x; 1.0349x over previous
"""Trainium2 Bass kernel for nn_LinearTransformerLayer_44495861187342.

Reference network: SGIRA block (self-attn MHA-16h -> LN -> cross-attn -> LN ->
gate blend -> FFN(gelu) -> LN) followed by a SAIGA block (squeeze-excite MLP ->
LN -> 4-head self-attn -> LN).  With the shipped inputs gate == 1.0, so the
cross-attention branch is algebraically dead and memory_states is unused; a
general path that includes it is kept for gate != 1.

Sharding (8 NeuronCores): core c owns 512 rows = (batch c//2, half c%2) of the
[4, 1024, 1024] input.  Row-local ops (projections, FFN, layernorm, softmax)
shard perfectly.  SGIRA self-attention K/V are recomputed per core for the full
1024-row batch (input replicated host-side).  SAIGA K/V are computed for the
own 512 rows only and exchanged within each core pair via one AllReduce(add);
the peer half is recovered as sum - own, which overlaps the collective with the
own-half attention work.

Layout: activations are feature-major in SBUF ([feat partitions, rows free]) so
matmuls contract the partition dim against natural-layout weights, and
layernorm/softmax feature reductions are PE ones-matmuls.  All matmul operands
are bf16 (fp32 accumulation in PSUM); LN statistics and the final output are
fp32.  Scalar engine handles exp/gelu; all other PSUM->SBUF copies run on the
vector engine with the bias folded in.
"""

import contextlib

import ml_dtypes
import numpy as np

import concourse.bass as bass
import concourse.mybir as mybir
import concourse.tile as tile
from concourse import bacc
from concourse import bass_utils

F32 = mybir.dt.float32
F32R = mybir.dt.float32r
BF16 = mybir.dt.bfloat16
F8 = mybir.dt.float8e4
DR = mybir.MatmulPerfMode.DoubleRow
AF = mybir.ActivationFunctionType
OP = mybir.AluOpType
WSC = 32.0       # fp8 weight pre-scale (descale folded into psum copies)

D = 1024          # model dim
DFF = 4096        # ffn dim
D2 = 2048         # squeeze-excite dim
S = 1024          # full sequence rows per batch
R = 512           # rows owned per core
P = 128
C = D // P        # 8 feature chunks
CFF = DFF // P    # 32
C2 = D2 // P      # 16
H_SA = 16         # SGIRA heads (hd 64)
H_SG = 4          # SAIGA heads (hd 256)
N_CORES = 8
EPS = 1e-5

_CACHE = {}
_DBG_TAPS = False  # set by dbg script only: dumps ss/hidden/se intermediates


def _build(include_cross: bool, with_vbias: bool):
    nc = bacc.Bacc("TRN2", target_bir_lowering=False, debug=False,
                   num_devices=N_CORES)

    def din(name, shape, dt=BF16):
        return nc.dram_tensor(name, shape, dt, kind="ExternalInput")

    # feature-major inputs (host pre-transposed), own 512 rows first
    xT = din("xT", [D, S])
    wq = din("wq", [D, D]); wk = din("wk", [D, D]); wv = din("wv", [D, D])
    bq = din("bq", [P, C], F32); bk = din("bk", [P, C], F32)
    wo = din("wo", [D, D]); bo = din("bo", [P, C], F32)
    w1 = din("w1", [D, DFF]); b1 = din("b1", [P, CFF], F32)
    w2 = din("w2", [DFF, D]); b2 = din("b2", [P, C], F32)
    exw = din("exw", [D, D2]); exb = din("exb", [P, C2], F32)
    sqw = din("sqw", [D2, D]); sqb = din("sqb", [P, C], F32)
    qw = din("qw", [D, D]); qb = din("qb", [P, C], F32)
    kw = din("kw", [D, D]); kb = din("kb", [P, C], F32)
    vw = din("vw", [D, D])
    nsg = din("nsg", [P, C], F32); nsb = din("nsb", [P, C], F32)
    nfg = din("nfg", [P, C], F32); nfb = din("nfb", [P, C], F32)
    nrg = din("nrg", [P, C], F32); nrb = din("nrb", [P, C], F32)
    if with_vbias:
        bv = din("bv", [1, D])
        vb = din("vb", [1, D])
    if include_cross:
        mT = din("mT", [D, S])
        cwq = din("cwq", [D, D]); cwk = din("cwk", [D, D]); cwv = din("cwv", [D, D])
        cbq = din("cbq", [P, C], F32); cbk = din("cbk", [P, C], F32)
        cwo = din("cwo", [D, D]); cbo = din("cbo", [P, C], F32)
        ncg = din("ncg", [P, C], F32); ncb = din("ncb", [P, C], F32)
        gate_c = din("gate_c", [P, 1], F32)      # broadcast gate
        gate_1mc = din("gate_1mc", [P, 1], F32)  # broadcast (1 - gate)
        if with_vbias:
            cbv = din("cbv", [1, D])

    out_d = nc.dram_tensor("out", [D, R], F32, kind="ExternalOutput")
    if _DBG_TAPS:
        tap_d = {nm: nc.dram_tensor(f"tap_{nm}", [D, R], F32,
                                    kind="ExternalOutput")
                 for nm in ("ctxsa", "ss", "hidden", "se", "ctx2")}

    with tile.TileContext(nc) as tc:
        with contextlib.ExitStack() as ctx, \
             nc.allow_low_precision("bf16 operands feeding the PE"):
            const = ctx.enter_context(tc.tile_pool(name="const", bufs=1))
            wpool = ctx.enter_context(tc.tile_pool(name="wpool", bufs=4))
            # PSUM: mm 2 + score 4 + ctx 2 = 8 banks
            psp = ctx.enter_context(tc.tile_pool(name="psp", bufs=1,
                                                 space="PSUM"))
            small = ctx.enter_context(tc.tile_pool(name="small", bufs=2))

            # PSUM budget: mm 2x[P,R] + score 2x[P,2R] + ctx 2x[P,R] = 8 banks
            def ps_mm(name="psmm"):
                return psp.tile([P, R], F32, tag="mm", bufs=2, name=name)

            def ps_score(name="score"):
                return psp.tile([P, R], F32, tag="score", bufs=2, name=name)

            def ps_score2(name="score2"):
                return psp.tile([P, 2 * R], F32, tag="score", bufs=2,
                                name=name)

            def ps_ctx(name="ctxps"):
                return psp.tile([P, R], F32, tag="ctx", bufs=2, name=name)

            ones_col = const.tile([P, 1], BF16, tag="ones_col")
            nc.vector.memset(ones_col[:], 1.0)
            ones_row = const.tile([1, P], BF16, tag="ones_row")
            nc.vector.memset(ones_row[:], 1.0)
            ones_f = const.tile([1, P], F32, tag="ones_f")
            nc.vector.memset(ones_f[:], 1.0)
            ones_row_r = const.tile([1, P], F32R, tag="ones_row_r")
            nc.scalar.copy(ones_row_r[:], ones_f[:])
            eps_t = const.tile([1, 1], F32, tag="eps")
            nc.vector.memset(eps_t[:], EPS)

            from concourse.dve_ops import (
                RECIP_APPROX_FAST_CONSTS,
                RECIPROCAL_APPROX_FAST,
            )

            def recip_r(out_r, in_ap):
                """~18-bit 1/x straight into an f32r-typed tile (single DVE
                op; the f32r output dtype satisfies the BIR verifier for
                downstream f32r matmuls)."""
                c = RECIP_APPROX_FAST_CONSTS
                nc.vector._custom_dve(
                    RECIPROCAL_APPROX_FAST, out=out_r, in0=in_ap,
                    s0=c["s0"], s1=c["s1"], imm2=c["imm2"])

            # HAM warmup: ~5us of dummy matmuls so the PE clock is at 2.4GHz
            # by the time the first real matmul's inputs arrive from HBM.
            warm = const.tile([P, R], BF16, tag="warm")
            nc.vector.memset(warm[:], 0.001)
            wu_ps = psp.tile([P, R], F32, tag="ctx", bufs=2, name="warmups")
            for i in range(28):
                nc.tensor.matmul(wu_ps[:], warm[:, 0:P], warm[:],
                                 start=(i == 0), stop=(i == 27))
            wu_sb = const.tile([1, 1], F32, tag="warmsb")
            nc.vector.tensor_scalar(wu_sb[:], wu_ps[0:1, 0:1], 0.0, None,
                                    OP.add)

            # Dummy pairwise collective early in the kernel: initializes the
            # CC rings and aligns the pair so the real SAIGA AllReduce does
            # not pay cold-start/skew latency.
            dram0 = ctx.enter_context(tc.tile_pool(name="dram0", bufs=1,
                                                   space="DRAM"))
            ccd_in = dram0.tile([1, 16], F32, tag="ccd_in")
            ccd_out = dram0.tile([1, 16], F32, tag="ccd_out")
            nc.gpsimd.dma_start(out=ccd_in[:], in_=ones_f[0:1, 0:16])
            nc.gpsimd.collective_compute(
                "AllReduce", OP.add,
                replica_groups=[[0, 1], [2, 3], [4, 5], [6, 7]],
                ins=[ccd_in.opt()], outs=[ccd_out.opt()])

            def bcast_rows(rec_r):
                """[1, R] f32r -> [P, R] psum broadcast via an f32r matmul."""
                bc_ps = ps_score("bcast")
                nc.tensor.matmul(bc_ps[:], ones_row_r[:], rec_r[:],
                                 start=True, stop=True)
                return bc_ps

            def load_pc(dram):  # small per-partition tables
                t = const.tile(list(dram.shape), dram.dtype,
                               tag=dram.name + "_sb")
                nc.sync.dma_start(out=t[:], in_=dram.ap())
                return t

            def wstat_ap(w, oc, kcs):
                # [P, kcs, 128] stationary block: w[(kc p), oc*128 + m]
                return w.ap().rearrange("(k p) m -> p k m", p=P)[:, :, oc * P:(oc + 1) * P]

            def tap(nm, t):
                if _DBG_TAPS:
                    f32t = const.tile([P, C, R], F32, tag="tapbuf")
                    for c in range(C):
                        nc.vector.tensor_scalar(f32t[:, c, :], t[:, c, :],
                                                0.0, None, OP.add)
                    nc.sync.dma_start(
                        out=tap_d[nm].ap().rearrange("(c p) r -> p c r", p=P),
                        in_=f32t[:])

            def copy_bias(out_ap, ps, bias_s, idx=0):
                """PSUM -> SBUF copy with per-partition bias; alternates
                between DVE and ACT so neither engine gates the PE."""
                if idx % 2 == 0:
                    nc.vector.tensor_scalar(out_ap, ps, bias_s, None, OP.add)
                else:
                    nc.scalar.activation(out_ap, ps, AF.Identity, bias=bias_s)

            def proj(out_t, out_c, in_t, in_c, w, bias_s, func=None):
                """out_t[:, oc, r] = func(sum_k w[k, oc*128+p] * in_t[k, r] + b)

                over the own R rows of in_t."""
                for oc in range(out_c):
                    wt = wpool.tile([P, in_c, P], BF16, tag="wstat",
                                    name=f"wst_{w.name}{oc}")
                    nc.sync.dma_start(out=wt[:], in_=wstat_ap(w, oc, in_c))
                    ps = ps_mm()
                    for kc in range(in_c):
                        nc.tensor.matmul(ps[:], wt[:, kc, :], in_t[:, kc, 0:R],
                                         start=(kc == 0), stop=(kc == in_c - 1))
                    if func == "gelu":
                        nc.scalar.activation(out_t[:, oc, :], ps[:], AF.Gelu,
                                             bias=bias_s[:, oc:oc + 1])
                    elif func == "relu":
                        nc.vector.tensor_scalar(out_t[:, oc, :], ps[:],
                                                bias_s[:, oc:oc + 1], 0.0,
                                                OP.add, OP.max)
                    else:
                        copy_bias(out_t[:, oc, :], ps[:], bias_s[:, oc:oc + 1],
                                  oc)

            def stage8(src_t, n_c, pool, tag):
                """bf16 [P, n_c, R] -> fp8 copy for DoubleRow matmul moving
                operands."""
                t8 = pool.tile([P, n_c, R], F8, tag=tag)
                for c in range(n_c):
                    if c % 2 == 0:
                        nc.vector.tensor_scalar(t8[:, c, :], src_t[:, c, :],
                                                0.0, None, OP.add)
                    else:
                        nc.scalar.activation(t8[:, c, :], src_t[:, c, :],
                                             AF.Copy)
                return t8

            def proj8(out_t, out_c, in8_t, in_c, w, bias_s, func=None,
                      wtag="wstat8", wpool_=None):
                """fp8 DoubleRow projection: weights pre-scaled by WSC on the
                host; 1/WSC folded into the psum->SBUF copy."""
                wp = wpool_ or wpool
                for oc in range(out_c):
                    wt = wp.tile([P, in_c, P], F8, tag=wtag,
                                 name=f"w8_{w.name}{oc}")
                    nc.sync.dma_start(out=wt[:], in_=wstat_ap(w, oc, in_c))
                    ps = ps_mm()
                    for kc in range(0, in_c, 2):
                        nc.tensor.matmul(ps[:], wt[:, kc:kc + 2, :],
                                         in8_t[:, kc:kc + 2, :],
                                         start=(kc == 0),
                                         stop=(kc == in_c - 2), perf_mode=DR)
                    if func == "gelu":
                        nc.scalar.activation(out_t[:, oc, :], ps[:], AF.Gelu,
                                             bias=bias_s[:, oc:oc + 1],
                                             scale=1.0 / WSC)
                    elif func == "relu":
                        nc.scalar.activation(out_t[:, oc, :], ps[:], AF.Relu,
                                             bias=bias_s[:, oc:oc + 1],
                                             scale=1.0 / WSC)
                    elif oc % 2 == 0:
                        nc.vector.tensor_scalar(out_t[:, oc, :], ps[:],
                                                1.0 / WSC,
                                                bias_s[:, oc:oc + 1],
                                                OP.mult, OP.add)
                    else:
                        nc.scalar.activation(out_t[:, oc, :], ps[:],
                                             AF.Identity,
                                             bias=bias_s[:, oc:oc + 1],
                                             scale=1.0 / WSC)

            def layer_norm(a_t, n_c, g_s, b_s, out_t):
                """Row-wise LN over the (P * n_c) feature dim of a_t [P,n_c,R]."""
                inv_n = 1.0 / (n_c * P)
                ps_sum = ps_score("lnsum")
                ps_sq = ps_score("lnsumsq")
                for c in range(n_c):
                    sqc = small.tile([P, R], BF16, tag="lnsqc", name="lnsqc")
                    nc.vector.tensor_mul(sqc[:], a_t[:, c, :], a_t[:, c, :])
                    nc.tensor.matmul(ps_sum[0:1, :], ones_col[:], a_t[:, c, :],
                                     start=(c == 0), stop=(c == n_c - 1))
                    nc.tensor.matmul(ps_sq[0:1, :], ones_col[:], sqc[:],
                                     start=(c == 0), stop=(c == n_c - 1))
                mu = small.tile([1, R], F32R, tag="lnmu", name="lnmu")
                nc.scalar.activation(mu[:], ps_sum[0:1, :], AF.Copy,
                                     scale=inv_n)
                va = small.tile([1, R], F32, tag="lnva", name="lnva")
                nc.scalar.activation(va[:], ps_sq[0:1, :], AF.Copy,
                                     scale=inv_n)
                mu2 = small.tile([1, R], F32, tag="lnmu2", name="lnmu2")
                nc.vector.tensor_mul(mu2[:], mu[:].bitcast(F32),
                                     mu[:].bitcast(F32))
                nc.vector.tensor_sub(va[:], va[:], mu2[:])
                nc.scalar.activation(va[:], va[:], AF.Sqrt, bias=eps_t[:])
                rstd = small.tile([1, R], F32R, tag="lnrstd", name="lnrstd")
                recip_r(rstd[:], va[:])
                # out = ((x - mu_bc) * rstd_bc) * g + b
                bcb_ps = bcast_rows(mu)     # broadcast mu
                bca_ps = bcast_rows(rstd)   # broadcast rstd
                bcb = small.tile([P, R], BF16, tag="lnbcbs", name="lnbcbs")
                nc.scalar.activation(bcb[:], bcb_ps[:], AF.Copy)
                bca = small.tile([P, R], BF16, tag="lnbcas", name="lnbcas")
                nc.vector.tensor_scalar(bca[:], bca_ps[:], 0.0, None, OP.add)
                for c in range(n_c):
                    nc.vector.tensor_sub(out_t[:, c, :], a_t[:, c, :], bcb[:])
                    nc.vector.tensor_mul(out_t[:, c, :], out_t[:, c, :],
                                         bca[:])
                    nc.vector.tensor_scalar(
                        out_t[:, c, :], out_t[:, c, :],
                        g_s[:, c:c + 1], b_s[:, c:c + 1], OP.mult, OP.add)

            def vproj_16(kv_src, w_v, vbias_t, vpool, wmpool, rcs, tag):
                """Row-major V for 16-head attention: [P, rc, 16, 65] with a
                ones column per head (softmax denominator trick)."""
                v4 = vpool.tile([P, len(rcs), H_SA, 65], BF16, tag=tag)
                nc.vector.memset(v4[:, :, :, 64:65], 1.0)
                for qh in range(2):
                    wm = wmpool.tile([P, C, R], BF16, tag="wmov",
                                     name=f"wm_{tag}{qh}")
                    nc.sync.dma_start(
                        out=wm[:],
                        in_=w_v.ap().rearrange("(k p) m -> p k m", p=P)
                        [:, :, qh * R:(qh + 1) * R])
                    for i, rc in enumerate(rcs):
                        ps = ps_mm()
                        for kc in range(C):
                            nc.tensor.matmul(
                                ps[:], kv_src[:, kc, rc * P:(rc + 1) * P],
                                wm[:, kc, :], start=(kc == 0),
                                stop=(kc == C - 1 and vbias_t is None))
                        if vbias_t is not None:
                            nc.tensor.matmul(
                                ps[:], ones_row[:],
                                vbias_t[0:1, qh * R:(qh + 1) * R],
                                start=False, stop=True)
                        if rc % 2 == 0:
                            nc.vector.tensor_scalar(
                                v4[:, i, qh * 8:(qh + 1) * 8, 0:64],
                                ps.rearrange("p (h e) -> p h e", e=64),
                                0.0, None, OP.add)
                        else:
                            nc.scalar.activation(
                                v4[:, i, qh * 8:(qh + 1) * 8, 0:64],
                                ps.rearrange("p (h e) -> p h e", e=64),
                                AF.Copy)
                return v4

            def mha16(kv_src, q_src, w_q, w_k, w_v, bq_t, bk_t, bv_t, pools):
                """16-head attention; q over own R rows, k/v over S rows of
                kv_src.  Returns ctx feature-major [P, C, R]."""
                apool, vpool, kqpool, expool, wmpool = pools
                v4 = vproj_16(kv_src, w_v, bv_t, vpool, wmpool,
                              list(range(C)), "v_sa")
                ctx_t = apool.tile([P, C, R], BF16, tag="ctx_sa",
                                   name="ctx_sa")
                kq = {}

                def kq_proj(oc):
                    wt = wpool.tile([P, C, P], BF16, tag="wstat",
                                    name=f"wstk{oc}")
                    nc.sync.dma_start(out=wt[:], in_=wstat_ap(w_k, oc, C))
                    kf = kqpool.tile([P, S], BF16, tag="kf", name="kf")
                    for rh in range(2):
                        ps = ps_mm()
                        for kc in range(C):
                            nc.tensor.matmul(
                                ps[:], wt[:, kc, :],
                                kv_src[:, kc, rh * R:(rh + 1) * R],
                                start=(kc == 0), stop=(kc == C - 1))
                        copy_bias(kf[:, rh * R:(rh + 1) * R], ps[:],
                                  bk_t[:, oc:oc + 1], rh)
                    wtq = wpool.tile([P, C, P], BF16, tag="wstat",
                                     name=f"wstq{oc}")
                    nc.sync.dma_start(out=wtq[:], in_=wstat_ap(w_q, oc, C))
                    qf = kqpool.tile([P, R], BF16, tag="qf", name="qf")
                    ps = ps_mm()
                    for kc in range(C):
                        nc.tensor.matmul(ps[:], wtq[:, kc, :],
                                         q_src[:, kc, 0:R],
                                         start=(kc == 0), stop=(kc == C - 1))
                    copy_bias(qf[:], ps[:], bq_t[:, oc:oc + 1], oc)
                    kq[oc] = (kf, qf)

                def att(oc):
                    kf, qf = kq.pop(oc)
                    ctx_ps = [ps_ctx(f"ctxps{hh}") for hh in range(2)]
                    for kc in range(C):
                        # row-tiled concurrent score pair (heads 2oc, 2oc+1)
                        # into the two banks of one [P, 2R] tile, then one
                        # batched exp for both heads
                        ps_s = ps_score2("scoreAB")
                        for hh in range(2):
                            po = hh * 64
                            nc.tensor.matmul(
                                ps_s[:, hh * R:(hh + 1) * R],
                                kf[po:po + 64, kc * P:(kc + 1) * P],
                                qf[po:po + 64, :], start=True, stop=True)
                        ex = expool.tile([P, 2 * R], BF16, tag="exp",
                                         name="exAB")
                        nc.scalar.activation(ex[:], ps_s[:], AF.Exp,
                                             scale=0.125)
                        for hh in range(2):
                            h = oc * 2 + hh
                            nc.tensor.matmul(
                                ctx_ps[hh][:65, :],
                                v4[:, kc, h, 0:65],
                                ex[:, hh * R:(hh + 1) * R],
                                start=(kc == 0), stop=(kc == C - 1))
                    for hh in range(2):
                        po = hh * 64
                        # custom-DVE ops misread PSUM at partition offsets;
                        # stage the denominator row through SBUF first
                        den = small.tile([1, R], F32, tag="attden",
                                         name="attden")
                        nc.scalar.activation(den[:], ctx_ps[hh][64:65, :],
                                             AF.Copy)
                        rec = small.tile([1, R], F32R, tag="rec", name="rec")
                        recip_r(rec[:], den[:])
                        bc_ps = ps_score("bcast")
                        nc.tensor.matmul(bc_ps[0:64, :],
                                         ones_row_r[0:1, 0:64], rec[:],
                                         start=True, stop=True)
                        bc_sb = small.tile([64, R], BF16, tag="bcsb",
                                           name="bcsb")
                        nc.scalar.activation(bc_sb[:], bc_ps[0:64, :],
                                             AF.Copy)
                        nc.vector.tensor_mul(ctx_t[po:po + 64, oc, :],
                                             ctx_ps[hh][0:64, :], bc_sb[:])

                kq_proj(0)
                kq_proj(1)
                for oc in range(C):
                    if oc + 2 < C:
                        kq_proj(oc + 2)
                    att(oc)
                return ctx_t

            # ---------------- phase 1: SGIRA self-attention ----------------
            ssp_cm = tc.tile_pool(name="ssp", bufs=1, side="right")
            ssp = ssp_cm.__enter__()
            ss = ssp.tile([P, C, R], BF16, tag="ss")
            with tc.tile_pool(name="p1", bufs=1) as p1, \
                 tc.tile_pool(name="p1kq", bufs=2) as p1kq, \
                 tc.tile_pool(name="p1ex", bufs=3) as p1ex, \
                 tc.tile_pool(name="p1wm", bufs=2) as p1wm:
                xT_s = p1.tile([P, C, S], BF16, tag="xT")
                xt_ap = xT.ap().rearrange("(c p) r -> p c r", p=P)
                for c in range(C):
                    nc.sync.dma_start(out=xT_s[:, c, :], in_=xt_ap[:, c, :])
                bq_s = load_pc(bq); bk_s = load_pc(bk)
                bv_s = load_pc(bv) if with_vbias else None
                ctx_sa = mha16(xT_s, xT_s, wq, wk, wv, bq_s, bk_s, bv_s,
                               (p1, p1, p1kq, p1ex, p1wm))
                # out-proj + residual + LN -> ss
                bo_s = load_pc(bo)
                nsg_s = load_pc(nsg); nsb_s = load_pc(nsb)
                tap("ctxsa", ctx_sa)
                sa = p1.tile([P, C, R], BF16, tag="sa")
                proj(sa, C, ctx_sa, C, wo, bo_s)
                for c in range(C):
                    nc.vector.tensor_add(sa[:, c, :], sa[:, c, :],
                                         xT_s[:, c, 0:R])
                layer_norm(sa, C, nsg_s, nsb_s, ss)
                tap("ss", ss)

            # ---------------- optional cross-attention (gate != 1) ---------
            if include_cross:
                fusedp_cm = tc.tile_pool(name="fusedp", bufs=1, side="right")
                fusedp = fusedp_cm.__enter__()
                fused = fusedp.tile([P, C, R], BF16, tag="fused")
                with tc.tile_pool(name="pc1", bufs=1) as pc1, \
                     tc.tile_pool(name="pc1kq", bufs=2) as pc1kq, \
                     tc.tile_pool(name="pc1ex", bufs=3) as pc1ex, \
                     tc.tile_pool(name="pc1wm", bufs=2) as pc1wm:
                    mT_s = pc1.tile([P, C, S], BF16, tag="mT")
                    mt_ap = mT.ap().rearrange("(c p) r -> p c r", p=P)
                    for c in range(C):
                        nc.sync.dma_start(out=mT_s[:, c, :], in_=mt_ap[:, c, :])
                    cbq_s = load_pc(cbq); cbk_s = load_pc(cbk)
                    cbv_s = load_pc(cbv) if with_vbias else None
                    ctx_ca = mha16(mT_s, ss, cwq, cwk, cwv, cbq_s, cbk_s,
                                   cbv_s, (pc1, pc1, pc1kq, pc1ex, pc1wm))
                    cbo_s = load_pc(cbo)
                    ncg_s = load_pc(ncg); ncb_s = load_pc(ncb)
                    gc_s = load_pc(gate_c); g1_s = load_pc(gate_1mc)
                    ca = pc1.tile([P, C, R], BF16, tag="ca")
                    proj(ca, C, ctx_ca, C, cwo, cbo_s)
                    for c in range(C):
                        nc.vector.tensor_add(ca[:, c, :], ca[:, c, :],
                                             ss[:, c, :])
                    cs = pc1.tile([P, C, R], BF16, tag="cs")
                    layer_norm(ca, C, ncg_s, ncb_s, cs)
                    # fused = gate*ss + (1-gate)*cs
                    for c in range(C):
                        nc.vector.tensor_scalar(
                            fused[:, c, :], ss[:, c, :], gc_s[:, 0:1], None,
                            OP.mult)
                        nc.vector.tensor_scalar(
                            cs[:, c, :], cs[:, c, :], g1_s[:, 0:1], None,
                            OP.mult)
                        nc.vector.tensor_add(fused[:, c, :], fused[:, c, :],
                                             cs[:, c, :])
                ff_in = fused
            else:
                ff_in = ss

            # ---------------- phase 2: FFN ----------------
            hidp_cm = tc.tile_pool(name="hidp", bufs=1)
            hidp = hidp_cm.__enter__()
            hidden = hidp.tile([P, C, R], BF16, tag="hidden")
            with tc.tile_pool(name="p2", bufs=1) as p2:
                b1_s = load_pc(b1); b2_s = load_pc(b2)
                nfg_s = load_pc(nfg); nfb_s = load_pc(nfb)
                h1 = p2.tile([P, CFF, R], BF16, tag="h1")
                proj(h1, CFF, ff_in, C, w1, b1_s, func="gelu")
                ffo = p2.tile([P, C, R], BF16, tag="ffo")
                for oc in range(C):
                    wt2 = p2.tile([P, CFF, P], BF16, tag="wstat2", bufs=3,
                                  name=f"wst2_{oc}")
                    nc.sync.dma_start(out=wt2[:], in_=wstat_ap(w2, oc, CFF))
                    ps = ps_mm()
                    for kc in range(CFF):
                        nc.tensor.matmul(ps[:], wt2[:, kc, :], h1[:, kc, :],
                                         start=(kc == 0), stop=(kc == CFF - 1))
                    copy_bias(ffo[:, oc, :], ps[:], b2_s[:, oc:oc + 1], oc)
                for c in range(C):
                    nc.vector.tensor_add(ffo[:, c, :], ffo[:, c, :],
                                         ff_in[:, c, :])
                layer_norm(ffo, C, nfg_s, nfb_s, hidden)
                tap("hidden", hidden)
            # ss (or fused) no longer needed
            if include_cross:
                fusedp_cm.__exit__(None, None, None)
            ssp_cm.__exit__(None, None, None)

            # ---------------- phase 3: SAIGA squeeze-excite ----------------
            sep_cm = tc.tile_pool(name="sep", bufs=1, side="right")
            sep = sep_cm.__enter__()
            se_own = sep.tile([P, C, R], BF16, tag="se_own")
            with tc.tile_pool(name="p3", bufs=1) as p3:
                exb_s = load_pc(exb); sqb_s = load_pc(sqb)
                nrg_s = load_pc(nrg); nrb_s = load_pc(nrb)
                h2 = p3.tile([P, C2, R], BF16, tag="h2")
                proj(h2, C2, hidden, C, exw, exb_s, func="relu")
                sqo = p3.tile([P, C, R], BF16, tag="sqo")
                proj(sqo, C, h2, C2, sqw, sqb_s)
                for c in range(C):
                    nc.vector.tensor_add(sqo[:, c, :], sqo[:, c, :],
                                         hidden[:, c, :])
                layer_norm(sqo, C, nrg_s, nrb_s, se_own)
                tap("se", se_own)
            hidp_cm.__exit__(None, None, None)

            # ------- phase 4: SAIGA K/V own-half + pair AllReduce ----------
            # ------- phase 5: 4-head attention (own half overlaps the cc) --
            with tc.tile_pool(name="p5", bufs=1) as p5, \
                 tc.tile_pool(name="p5ex", bufs=3) as p5ex, \
                 tc.tile_pool(name="p5wm", bufs=2) as p5wm, \
                 tc.tile_pool(name="dramp", bufs=1, space="DRAM") as dramp:
                qb_s = load_pc(qb); kb_s = load_pc(kb)
                vb_s = load_pc(vb) if with_vbias else None

                # K2 feature-major [P, C, S]; own rows first
                k2 = p5.tile([P, C, S], BF16, tag="k2")
                for oc in range(C):
                    wt = wpool.tile([P, C, P], BF16, tag="wstat",
                                    name=f"wstk2_{oc}")
                    nc.sync.dma_start(out=wt[:], in_=wstat_ap(kw, oc, C))
                    ps = ps_mm()
                    for kc in range(C):
                        nc.tensor.matmul(ps[:], wt[:, kc, :],
                                         se_own[:, kc, :],
                                         start=(kc == 0), stop=(kc == C - 1))
                    copy_bias(k2[:, oc, 0:R], ps[:], kb_s[:, oc:oc + 1], oc)
                in_b = dramp.tile([2, P, 8 * R], BF16, tag="cc_in")
                red = dramp.tile([2, P, 8 * R], BF16, tag="cc_out")
                nc.gpsimd.dma_start(
                    out=in_b[0].rearrange("p (c r) -> p c r", c=C),
                    in_=k2[:, :, 0:R])

                # V2 row-major [P, rc, D]; own row chunks 0-3
                v2 = p5.tile([P, C, D], BF16, tag="v2")
                for qh in range(2):
                    wm = p5wm.tile([P, C, R], BF16, tag="wmov",
                                   name=f"wmv2_{qh}")
                    nc.sync.dma_start(
                        out=wm[:],
                        in_=vw.ap().rearrange("(k p) m -> p k m", p=P)
                        [:, :, qh * R:(qh + 1) * R])
                    for rc in range(4):
                        ps = ps_mm()
                        for kc in range(C):
                            nc.tensor.matmul(
                                ps[:], se_own[:, kc, rc * P:(rc + 1) * P],
                                wm[:, kc, :], start=(kc == 0),
                                stop=(kc == C - 1 and vb_s is None))
                        if vb_s is not None:
                            nc.tensor.matmul(
                                ps[:], ones_row[:],
                                vb_s[0:1, qh * R:(qh + 1) * R],
                                start=False, stop=True)
                        if rc % 2 == 0:
                            nc.vector.tensor_scalar(
                                v2[:, rc, qh * R:(qh + 1) * R], ps[:],
                                0.0, None, OP.add)
                        else:
                            nc.scalar.activation(
                                v2[:, rc, qh * R:(qh + 1) * R], ps[:],
                                AF.Copy)
                    nc.gpsimd.dma_start(
                        out=in_b[1].rearrange("p (c r) -> p c r", c=4)
                        [:, :, qh * R:(qh + 1) * R],
                        in_=v2[:, 0:4, qh * R:(qh + 1) * R])
                nc.gpsimd.collective_compute(
                    "AllReduce", OP.add,
                    replica_groups=[[0, 1], [2, 3], [4, 5], [6, 7]],
                    ins=[in_b.opt()], outs=[red.opt()])

                # Q2 projections (independent of the collective)
                q2 = p5.tile([P, C, R], BF16, tag="q2")
                proj(q2, C, se_own, C, qw, qb_s)

                # own-half attention: heads accumulate ctx/denominator over
                # k-chunks 0-3 into SBUF, freeing PSUM before the peer half
                ctx_own = p5.tile([P, C, R], BF16, tag="ctx_own")
                sum_own = p5.tile([1, H_SG, R], F32, tag="sum_own")
                ex_t = {}

                def att2_half(h, rng, ctx_sb, sum_sb, prev_ctx, prev_sum):
                    ctx_ps = [ps_ctx(f"c2ps{mh}") for mh in range(2)]
                    sum_ps = ps_mm("a2sum")
                    for j, kc in enumerate(rng):
                        first, last = (j == 0), (j == len(rng) - 1)
                        ps_s = ps_score("score2")
                        for i in range(2):
                            oc = 2 * h + i
                            nc.tensor.matmul(
                                ps_s[:], k2[:, oc, kc * P:(kc + 1) * P],
                                q2[:, oc, :], start=(i == 0), stop=(i == 1))
                        ex = p5ex.tile([P, R], BF16, tag="exp", name="ex2")
                        nc.scalar.activation(ex[:], ps_s[:], AF.Exp,
                                             scale=0.0625)
                        nc.tensor.matmul(sum_ps[0:1, :], ones_col[:], ex[:],
                                         start=first, stop=last)
                        for mh in range(2):
                            nc.tensor.matmul(
                                ctx_ps[mh][:],
                                v2[:, kc, (h * 256 + mh * P):(h * 256 + (mh + 1) * P)],
                                ex[:], start=first, stop=last)
                    if prev_ctx is None:
                        nc.vector.tensor_scalar(
                            ctx_sb[:, 2 * h, :], ctx_ps[0][:],
                            0.0, None, OP.add)
                        nc.scalar.activation(ctx_sb[:, 2 * h + 1, :],
                                             ctx_ps[1][:], AF.Copy)
                        nc.vector.tensor_scalar(sum_sb[0:1, h, :],
                                                sum_ps[0:1, :], 0.0, None,
                                                OP.add)
                    else:
                        # combine halves, normalize, write ctx
                        den = small.tile([1, R], F32, tag="a2den", name="den")
                        nc.vector.tensor_add(den[:], sum_ps[0:1, :],
                                             prev_sum[0:1, h, :])
                        rec = small.tile([1, R], F32R, tag="rec", name="rec2")
                        recip_r(rec[:], den[:])
                        bc_ps = bcast_rows(rec)
                        bc_sb = small.tile([P, R], BF16, tag="bcsb",
                                           name="bcsb2")
                        nc.scalar.activation(bc_sb[:], bc_ps[:], AF.Copy)
                        for mh in range(2):
                            oc = 2 * h + mh
                            tot = small.tile([P, R], BF16, tag="a2tot",
                                             name="tot")
                            nc.vector.tensor_add(tot[:], ctx_ps[mh][:],
                                                 prev_ctx[:, oc, :])
                            nc.vector.tensor_mul(ctx_sb[:, oc, :], tot[:],
                                                 bc_sb[:])

                for h in range(H_SG):
                    att2_half(h, range(4), ctx_own, sum_own, None, None)

                # peer recovery: peer = allreduce_sum - own
                ksum = p5.tile([P, C, R], BF16, tag="ksum")
                for half in range(2):
                    nc.sync.dma_start(
                        out=ksum[:, half * 4:(half + 1) * 4, :],
                        in_=red[0].rearrange("p (c r) -> p c r", c=C)
                        [:, half * 4:(half + 1) * 4, :])
                    for oc in range(half * 4, (half + 1) * 4):
                        nc.vector.tensor_sub(k2[:, oc, R:S], ksum[:, oc, :],
                                             k2[:, oc, 0:R])
                vsum = p5.tile([P, 4, D], BF16, tag="vsum")
                nc.sync.dma_start(
                    out=vsum[:],
                    in_=red[1].rearrange("p (c r) -> p c r", c=4))
                for rc in range(4):
                    nc.vector.tensor_sub(v2[:, 4 + rc, :], vsum[:, rc, :],
                                         v2[:, rc, :])

                ctx2 = p5.tile([P, C, R], BF16, tag="ctx2")
                for h in range(H_SG):
                    att2_half(h, range(4, 8), ctx2, None, ctx_own, sum_own)

                tap("ctx2", ctx2)
                # ---------------- phase 6: final residual + LN -------------
                for c in range(C):
                    nc.vector.tensor_add(ctx2[:, c, :], ctx2[:, c, :],
                                         se_own[:, c, :])
                fin = p5.tile([P, C, R], F32, tag="fin")
                layer_norm(ctx2, C, nrg_s, nrb_s, fin)
                nc.sync.dma_start(
                    out=out_d.ap().rearrange("(c p) r -> p c r", p=P),
                    in_=fin[:])
            sep_cm.__exit__(None, None, None)

    nc.compile()
    return nc


def _pc(v):
    """[n*128] -> [128, n] per-partition layout."""
    v = np.asarray(v, np.float32)
    return np.ascontiguousarray(v.reshape(-1, P).T)


def _bf(a):
    return np.ascontiguousarray(np.asarray(a, np.float32)
                                .astype(ml_dtypes.bfloat16))


def _f8(a):
    """fp8e4m3 weights pre-scaled by WSC (clipped to the TRN ±240 range)."""
    v = np.asarray(a, np.float32) * WSC
    return np.ascontiguousarray(np.clip(v, -240.0, 240.0)
                                .astype(ml_dtypes.float8_e4m3))


def kernel(**inputs):
    x = np.asarray(inputs["input_states"], np.float32)
    gate = float(np.asarray(inputs["gate"]).ravel()[0])
    include_cross = (gate != 1.0)

    bq, bk, bv = np.split(np.asarray(inputs["sa_in_b"], np.float32), 3)
    vb = np.asarray(inputs["v_b"], np.float32)
    cbv = (np.split(np.asarray(inputs["ca_in_b"], np.float32), 3)[2]
           if include_cross else np.zeros(1, np.float32))
    with_vbias = bool(np.any(bv) or np.any(vb) or np.any(cbv))

    key = (include_cross, with_vbias)
    if key not in _CACHE:
        _CACHE[key] = _build(include_cross, with_vbias)
    nc = _CACHE[key]

    wq, wk, wv = [_bf(w) for w in
                  np.split(np.asarray(inputs["sa_in_w"], np.float32), 3,
                           axis=1)]

    shared = {
        "wq": wq, "wk": wk, "wv": wv,
        "bq": _pc(bq), "bk": _pc(bk),
        "wo": _bf(inputs["sa_out_w"]),
        "bo": _pc(inputs["sa_out_b"]),
        "w1": _bf(inputs["ffn_w1"]),
        "b1": _pc(inputs["ffn_b1"]),
        "w2": _bf(inputs["ffn_w2"]),
        "b2": _pc(inputs["ffn_b2"]),
        "exw": _bf(inputs["ex_w"]),
        "exb": _pc(inputs["ex_b"]),
        "sqw": _bf(inputs["sq_w"]),
        "sqb": _pc(inputs["sq_b"]),
        "qw": _bf(inputs["q_w"]),
        "qb": _pc(inputs["q_b"]),
        "kw": _bf(inputs["k_w"]),
        "kb": _pc(inputs["k_b"]),
        "vw": _bf(inputs["v_w"]),
        "nsg": _pc(inputs["ns_g"]), "nsb": _pc(inputs["ns_b"]),
        "nfg": _pc(inputs["nf_g"]), "nfb": _pc(inputs["nf_b"]),
        "nrg": _pc(inputs["nrm_g"]), "nrb": _pc(inputs["nrm_b"]),
    }
    if with_vbias:
        shared["bv"] = _bf(bv.reshape(1, D))
        shared["vb"] = _bf(vb.reshape(1, D))
    if include_cross:
        m = np.asarray(inputs["memory_states"], np.float32)
        cwq, cwk, cwv = [_bf(w) for w in
                         np.split(np.asarray(inputs["ca_in_w"], np.float32),
                                  3, axis=1)]
        cbq, cbk, cbv_ = np.split(np.asarray(inputs["ca_in_b"], np.float32), 3)
        shared.update({
            "cwq": cwq, "cwk": cwk, "cwv": cwv,
            "cbq": _pc(cbq), "cbk": _pc(cbk),
            "cwo": _bf(inputs["ca_out_w"]),
            "cbo": _pc(inputs["ca_out_b"]),
            "ncg": _pc(inputs["nc_g"]), "ncb": _pc(inputs["nc_b"]),
            "gate_c": np.full((P, 1), gate, np.float32),
            "gate_1mc": np.full((P, 1), 1.0 - gate, np.float32),
        })
        if with_vbias:
            shared["cbv"] = _bf(cbv_.reshape(1, D))

    in_maps = []
    for c in range(N_CORES):
        b, hf = c // 2, c % 2
        xp = np.concatenate([x[b, hf * R:(hf + 1) * R],
                             x[b, (1 - hf) * R:(2 - hf) * R]], axis=0)
        m_in = dict(shared)
        m_in["xT"] = _bf(xp.T)
        if include_cross:
            m_in["mT"] = _bf(m[b].T)
        in_maps.append(m_in)

    res = bass_utils.run_bass_kernel_spmd(nc, in_maps,
                                          core_ids=list(range(N_CORES)))
    out = np.empty((4, S, D), np.float32)
    for c in range(N_CORES):
        b, hf = c // 2, c % 2
        out[b, hf * R:(hf + 1) * R, :] = res.results[c]["out"].T
    return out


# revision 58
# speedup vs baseline: 1.5834x; 1.0061x over previous
"""Trainium2 Bass kernel for nn_LinearTransformerLayer_44495861187342.

Reference network: SGIRA block (self-attn MHA-16h -> LN -> cross-attn -> LN ->
gate blend -> FFN(gelu) -> LN) followed by a SAIGA block (squeeze-excite MLP ->
LN -> 4-head self-attn -> LN).  With the shipped inputs gate == 1.0, so the
cross-attention branch is algebraically dead and memory_states is unused; a
general path that includes it is kept for gate != 1.

Sharding (8 NeuronCores): core c owns 512 rows = (batch c//2, half c%2) of the
[4, 1024, 1024] input.  Row-local ops (projections, FFN, layernorm, softmax)
shard perfectly.  SGIRA self-attention K/V are recomputed per core for the full
1024-row batch (input replicated host-side).  SAIGA K/V are computed for the
own 512 rows only and exchanged within each core pair via one AllReduce(add);
the peer half is recovered as sum - own, which overlaps the collective with the
own-half attention work.

Layout: activations are feature-major in SBUF ([feat partitions, rows free]) so
matmuls contract the partition dim against natural-layout weights, and
layernorm/softmax feature reductions are PE ones-matmuls.  All matmul operands
are bf16 (fp32 accumulation in PSUM); LN statistics and the final output are
fp32.  Scalar engine handles exp/gelu; all other PSUM->SBUF copies run on the
vector engine with the bias folded in.
"""

import contextlib

import ml_dtypes
import numpy as np

import concourse.bass as bass
import concourse.mybir as mybir
import concourse.tile as tile
from concourse import bacc
from concourse import bass_utils

F32 = mybir.dt.float32
F32R = mybir.dt.float32r
BF16 = mybir.dt.bfloat16
F8 = mybir.dt.float8e4
DR = mybir.MatmulPerfMode.DoubleRow
AF = mybir.ActivationFunctionType
OP = mybir.AluOpType
WSC = 32.0       # fp8 weight pre-scale (descale folded into psum copies)

D = 1024          # model dim
DFF = 4096        # ffn dim
D2 = 2048         # squeeze-excite dim
S = 1024          # full sequence rows per batch
R = 512           # rows owned per core
P = 128
C = D // P        # 8 feature chunks
CFF = DFF // P    # 32
C2 = D2 // P      # 16
H_SA = 16         # SGIRA heads (hd 64)
H_SG = 4          # SAIGA heads (hd 256)
N_CORES = 8
EPS = 1e-5

_CACHE = {}
_DBG_TAPS = False  # set by dbg script only: dumps ss/hidden/se intermediates


def _build(include_cross: bool, with_vbias: bool):
    nc = bacc.Bacc("TRN2", target_bir_lowering=False, debug=False,
                   num_devices=N_CORES)

    def din(name, shape, dt=BF16):
        return nc.dram_tensor(name, shape, dt, kind="ExternalInput")

    # feature-major inputs (host pre-transposed), own 512 rows first
    xT = din("xT", [D, S])
    wq = din("wq", [D, D]); wk = din("wk", [D, D]); wv = din("wv", [D, D])
    bq = din("bq", [P, C], F32); bk = din("bk", [P, C], F32)
    wo = din("wo", [D, D]); bo = din("bo", [P, C], F32)
    w1 = din("w1", [D, DFF]); b1 = din("b1", [P, CFF], F32)
    w2 = din("w2", [DFF, D]); b2 = din("b2", [P, C], F32)
    exw = din("exw", [D, D2]); exb = din("exb", [P, C2], F32)
    sqw = din("sqw", [D2, D]); sqb = din("sqb", [P, C], F32)
    qw = din("qw", [D, D]); qb = din("qb", [P, C], F32)
    kw = din("kw", [D, D]); kb = din("kb", [P, C], F32)
    vw = din("vw", [D, D])
    nsg = din("nsg", [P, C], F32); nsb = din("nsb", [P, C], F32)
    nfg = din("nfg", [P, C], F32); nfb = din("nfb", [P, C], F32)
    nrg = din("nrg", [P, C], F32); nrb = din("nrb", [P, C], F32)
    if with_vbias:
        bv = din("bv", [1, D])
        vb = din("vb", [1, D])
    if include_cross:
        mT = din("mT", [D, S])
        cwq = din("cwq", [D, D]); cwk = din("cwk", [D, D]); cwv = din("cwv", [D, D])
        cbq = din("cbq", [P, C], F32); cbk = din("cbk", [P, C], F32)
        cwo = din("cwo", [D, D]); cbo = din("cbo", [P, C], F32)
        ncg = din("ncg", [P, C], F32); ncb = din("ncb", [P, C], F32)
        gate_c = din("gate_c", [P, 1], F32)      # broadcast gate
        gate_1mc = din("gate_1mc", [P, 1], F32)  # broadcast (1 - gate)
        if with_vbias:
            cbv = din("cbv", [1, D])

    out_d = nc.dram_tensor("out", [D, R], F32, kind="ExternalOutput")
    if _DBG_TAPS:
        tap_d = {nm: nc.dram_tensor(f"tap_{nm}", [D, R], F32,
                                    kind="ExternalOutput")
                 for nm in ("ctxsa", "ss", "hidden", "se", "ctx2")}

    with tile.TileContext(nc) as tc:
        with contextlib.ExitStack() as ctx, \
             nc.allow_low_precision("bf16 operands feeding the PE"):
            const = ctx.enter_context(tc.tile_pool(name="const", bufs=1))
            wpool = ctx.enter_context(tc.tile_pool(name="wpool", bufs=4))
            # PSUM: mm 2 + score 4 + ctx 2 = 8 banks
            psp = ctx.enter_context(tc.tile_pool(name="psp", bufs=1,
                                                 space="PSUM"))
            small = ctx.enter_context(tc.tile_pool(name="small", bufs=2))

            # PSUM budget: mm 2x[P,R] + score 2x[P,2R] + ctx 2x[P,R] = 8 banks
            def ps_mm(name="psmm"):
                return psp.tile([P, R], F32, tag="mm", bufs=2, name=name)

            def ps_score(name="score"):
                return psp.tile([P, R], F32, tag="score", bufs=2, name=name)

            def ps_score2(name="score2"):
                return psp.tile([P, 2 * R], F32, tag="score", bufs=2,
                                name=name)

            def ps_ctx(name="ctxps"):
                return psp.tile([P, R], F32, tag="ctx", bufs=2, name=name)

            ones_col = const.tile([P, 1], BF16, tag="ones_col")
            nc.vector.memset(ones_col[:], 1.0)
            ones_row = const.tile([1, P], BF16, tag="ones_row")
            nc.vector.memset(ones_row[:], 1.0)
            ones_f = const.tile([1, P], F32, tag="ones_f")
            nc.vector.memset(ones_f[:], 1.0)
            ones_row_r = const.tile([1, P], F32R, tag="ones_row_r")
            nc.scalar.copy(ones_row_r[:], ones_f[:])
            eps_t = const.tile([1, 1], F32, tag="eps")
            nc.vector.memset(eps_t[:], EPS)

            from concourse.dve_ops import (
                RECIP_APPROX_FAST_CONSTS,
                RECIPROCAL_APPROX_FAST,
            )

            def recip_r(out_r, in_ap):
                """~18-bit 1/x straight into an f32r-typed tile (single DVE
                op; the f32r output dtype satisfies the BIR verifier for
                downstream f32r matmuls)."""
                c = RECIP_APPROX_FAST_CONSTS
                nc.vector._custom_dve(
                    RECIPROCAL_APPROX_FAST, out=out_r, in0=in_ap,
                    s0=c["s0"], s1=c["s1"], imm2=c["imm2"])

            # HAM warmup: ~5us of dummy matmuls so the PE clock is at 2.4GHz
            # by the time the first real matmul's inputs arrive from HBM.
            warm = const.tile([P, R], BF16, tag="warm")
            nc.vector.memset(warm[:], 0.001)
            def keepwarm(n, name="kw"):
                """Dependency-free PE matmuls that pad unavoidable PE waits so
                the HAM clock gate never re-throttles the array."""
                wps = psp.tile([P, R], F32, tag="ctx", bufs=2, name=name)
                for i in range(n):
                    nc.tensor.matmul(wps[:], warm[:, 0:P], warm[:],
                                     start=(i == 0), stop=(i == n - 1))
                t = small.tile([1, 1], F32, tag="kwsb", name="kwsb")
                nc.vector.tensor_scalar(t[:], wps[0:1, 0:1], 0.0, None,
                                        OP.add)

            keepwarm(52, "warmup")

            # Dummy pairwise collective early in the kernel: initializes the
            # CC rings and aligns the pair so the real SAIGA AllReduce does
            # not pay cold-start/skew latency.
            dram0 = ctx.enter_context(tc.tile_pool(name="dram0", bufs=1,
                                                   space="DRAM"))
            ccd_in = dram0.tile([1, 16], F32, tag="ccd_in")
            ccd_out = dram0.tile([1, 16], F32, tag="ccd_out")
            nc.gpsimd.dma_start(out=ccd_in[:], in_=ones_f[0:1, 0:16])
            nc.gpsimd.collective_compute(
                "AllReduce", OP.add,
                replica_groups=[[0, 1], [2, 3], [4, 5], [6, 7]],
                ins=[ccd_in.opt()], outs=[ccd_out.opt()])

            def bcast_rows(rec_r):
                """[1, R] f32r -> [P, R] psum broadcast via an f32r matmul."""
                bc_ps = ps_score("bcast")
                nc.tensor.matmul(bc_ps[:], ones_row_r[:], rec_r[:],
                                 start=True, stop=True)
                return bc_ps

            def load_pc(dram):  # small per-partition tables
                t = const.tile(list(dram.shape), dram.dtype,
                               tag=dram.name + "_sb")
                nc.sync.dma_start(out=t[:], in_=dram.ap())
                return t

            def wstat_ap(w, oc, kcs):
                # [P, kcs, 128] stationary block: w[(kc p), oc*128 + m]
                return w.ap().rearrange("(k p) m -> p k m", p=P)[:, :, oc * P:(oc + 1) * P]

            def tap(nm, t):
                if _DBG_TAPS:
                    f32t = const.tile([P, C, R], F32, tag="tapbuf")
                    for c in range(C):
                        nc.vector.tensor_scalar(f32t[:, c, :], t[:, c, :],
                                                0.0, None, OP.add)
                    nc.sync.dma_start(
                        out=tap_d[nm].ap().rearrange("(c p) r -> p c r", p=P),
                        in_=f32t[:])

            def copy_bias(out_ap, ps, bias_s, idx=0):
                """PSUM -> SBUF copy with per-partition bias; alternates
                between DVE and ACT so neither engine gates the PE."""
                if idx % 2 == 0:
                    nc.vector.tensor_scalar(out_ap, ps, bias_s, None, OP.add)
                else:
                    nc.scalar.activation(out_ap, ps, AF.Identity, bias=bias_s)

            def proj(out_t, out_c, in_t, in_c, w, bias_s, func=None):
                """out_t[:, oc, r] = func(sum_k w[k, oc*128+p] * in_t[k, r] + b)

                over the own R rows of in_t."""
                for oc in range(out_c):
                    wt = wpool.tile([P, in_c, P], BF16, tag="wstat",
                                    name=f"wst_{w.name}{oc}")
                    nc.sync.dma_start(out=wt[:], in_=wstat_ap(w, oc, in_c))
                    ps = ps_mm()
                    for kc in range(in_c):
                        nc.tensor.matmul(ps[:], wt[:, kc, :], in_t[:, kc, 0:R],
                                         start=(kc == 0), stop=(kc == in_c - 1))
                    if func == "gelu":
                        nc.scalar.activation(out_t[:, oc, :], ps[:], AF.Gelu,
                                             bias=bias_s[:, oc:oc + 1])
                    elif func == "relu":
                        nc.vector.tensor_scalar(out_t[:, oc, :], ps[:],
                                                bias_s[:, oc:oc + 1], 0.0,
                                                OP.add, OP.max)
                    else:
                        copy_bias(out_t[:, oc, :], ps[:], bias_s[:, oc:oc + 1],
                                  oc)

            def stage8(src_t, n_c, pool, tag):
                """bf16 [P, n_c, R] -> fp8 copy for DoubleRow matmul moving
                operands."""
                t8 = pool.tile([P, n_c, R], F8, tag=tag)
                for c in range(n_c):
                    if c % 2 == 0:
                        nc.vector.tensor_scalar(t8[:, c, :], src_t[:, c, :],
                                                0.0, None, OP.add)
                    else:
                        nc.scalar.activation(t8[:, c, :], src_t[:, c, :],
                                             AF.Copy)
                return t8

            def proj8(out_t, out_c, in8_t, in_c, w, bias_s, func=None,
                      wtag="wstat8", wpool_=None):
                """fp8 DoubleRow projection: weights pre-scaled by WSC on the
                host; 1/WSC folded into the psum->SBUF copy."""
                wp = wpool_ or wpool
                for oc in range(out_c):
                    wt = wp.tile([P, in_c, P], F8, tag=wtag,
                                 name=f"w8_{w.name}{oc}")
                    nc.sync.dma_start(out=wt[:], in_=wstat_ap(w, oc, in_c))
                    ps = ps_mm()
                    for kc in range(0, in_c, 2):
                        nc.tensor.matmul(ps[:], wt[:, kc:kc + 2, :],
                                         in8_t[:, kc:kc + 2, :],
                                         start=(kc == 0),
                                         stop=(kc == in_c - 2), perf_mode=DR)
                    if func == "gelu":
                        nc.scalar.activation(out_t[:, oc, :], ps[:], AF.Gelu,
                                             bias=bias_s[:, oc:oc + 1],
                                             scale=1.0 / WSC)
                    elif func == "relu":
                        nc.scalar.activation(out_t[:, oc, :], ps[:], AF.Relu,
                                             bias=bias_s[:, oc:oc + 1],
                                             scale=1.0 / WSC)
                    elif oc % 2 == 0:
                        nc.vector.tensor_scalar(out_t[:, oc, :], ps[:],
                                                1.0 / WSC,
                                                bias_s[:, oc:oc + 1],
                                                OP.mult, OP.add)
                    else:
                        nc.scalar.activation(out_t[:, oc, :], ps[:],
                                             AF.Identity,
                                             bias=bias_s[:, oc:oc + 1],
                                             scale=1.0 / WSC)

            def layer_norm(a_t, n_c, g_s, b_s, out_t):
                """Row-wise LN over the (P * n_c) feature dim of a_t [P,n_c,R]."""
                inv_n = 1.0 / (n_c * P)
                keepwarm(8, "kwln")
                ps_sum = ps_score("lnsum")
                ps_sq = ps_score("lnsumsq")
                for c in range(n_c):
                    sqc = small.tile([P, R], BF16, tag="lnsqc", name="lnsqc")
                    nc.vector.tensor_mul(sqc[:], a_t[:, c, :], a_t[:, c, :])
                    nc.tensor.matmul(ps_sum[0:1, :], ones_col[:], a_t[:, c, :],
                                     start=(c == 0), stop=(c == n_c - 1))
                    nc.tensor.matmul(ps_sq[0:1, :], ones_col[:], sqc[:],
                                     start=(c == 0), stop=(c == n_c - 1))
                mu = small.tile([1, R], F32R, tag="lnmu", name="lnmu")
                nc.scalar.activation(mu[:], ps_sum[0:1, :], AF.Copy,
                                     scale=inv_n)
                va = small.tile([1, R], F32, tag="lnva", name="lnva")
                nc.scalar.activation(va[:], ps_sq[0:1, :], AF.Copy,
                                     scale=inv_n)
                mu2 = small.tile([1, R], F32, tag="lnmu2", name="lnmu2")
                nc.vector.tensor_mul(mu2[:], mu[:].bitcast(F32),
                                     mu[:].bitcast(F32))
                nc.vector.tensor_sub(va[:], va[:], mu2[:])
                nc.scalar.activation(va[:], va[:], AF.Sqrt, bias=eps_t[:])
                rstd = small.tile([1, R], F32R, tag="lnrstd", name="lnrstd")
                recip_r(rstd[:], va[:])
                # out = ((x - mu_bc) * rstd_bc) * g + b
                bcb_ps = bcast_rows(mu)     # broadcast mu
                bca_ps = bcast_rows(rstd)   # broadcast rstd
                keepwarm(16, "kwln2")
                bcb = small.tile([P, R], BF16, tag="lnbcbs", name="lnbcbs")
                nc.scalar.activation(bcb[:], bcb_ps[:], AF.Copy)
                bca = small.tile([P, R], BF16, tag="lnbcas", name="lnbcas")
                nc.vector.tensor_scalar(bca[:], bca_ps[:], 0.0, None, OP.add)
                for c in range(n_c):
                    nc.vector.tensor_sub(out_t[:, c, :], a_t[:, c, :], bcb[:])
                    nc.vector.tensor_mul(out_t[:, c, :], out_t[:, c, :],
                                         bca[:])
                    nc.vector.tensor_scalar(
                        out_t[:, c, :], out_t[:, c, :],
                        g_s[:, c:c + 1], b_s[:, c:c + 1], OP.mult, OP.add)

            def vproj_16(kv_src, w_v, vbias_t, vpool, wmpool, rcs, tag):
                """Row-major V for 16-head attention: [P, rc, 16, 65] with a
                ones column per head (softmax denominator trick)."""
                v4 = vpool.tile([P, len(rcs), H_SA, 65], BF16, tag=tag)
                nc.vector.memset(v4[:, :, :, 64:65], 1.0)
                for qh in range(2):
                    wm = wmpool.tile([P, C, R], BF16, tag="wmov",
                                     name=f"wm_{tag}{qh}")
                    wv_ap = w_v.ap().rearrange("(k p) m -> p k m", p=P)
                    for kc in range(C):
                        nc.sync.dma_start(
                            out=wm[:, kc, :],
                            in_=wv_ap[:, kc, qh * R:(qh + 1) * R])
                    for i, rc in enumerate(rcs):
                        ps = ps_mm()
                        for kc in range(C):
                            nc.tensor.matmul(
                                ps[:], kv_src[:, kc, rc * P:(rc + 1) * P],
                                wm[:, kc, :], start=(kc == 0),
                                stop=(kc == C - 1 and vbias_t is None))
                        if vbias_t is not None:
                            nc.tensor.matmul(
                                ps[:], ones_row[:],
                                vbias_t[0:1, qh * R:(qh + 1) * R],
                                start=False, stop=True)
                        if rc % 2 == 0:
                            nc.vector.tensor_scalar(
                                v4[:, i, qh * 8:(qh + 1) * 8, 0:64],
                                ps.rearrange("p (h e) -> p h e", e=64),
                                0.0, None, OP.add)
                        else:
                            nc.scalar.activation(
                                v4[:, i, qh * 8:(qh + 1) * 8, 0:64],
                                ps.rearrange("p (h e) -> p h e", e=64),
                                AF.Copy)
                return v4

            def mha16(kv_src, q_src, w_q, w_k, w_v, bq_t, bk_t, bv_t, pools):
                """16-head attention; q over own R rows, k/v over S rows of
                kv_src.  Returns ctx feature-major [P, C, R]."""
                apool, vpool, kqpool, expool, wmpool = pools
                v4 = vproj_16(kv_src, w_v, bv_t, vpool, wmpool,
                              list(range(C)), "v_sa")
                ctx_t = apool.tile([P, C, R], BF16, tag="ctx_sa",
                                   name="ctx_sa")
                kq = {}

                def kq_proj(oc):
                    wt = wpool.tile([P, C, P], BF16, tag="wstat",
                                    name=f"wstk{oc}")
                    nc.sync.dma_start(out=wt[:], in_=wstat_ap(w_k, oc, C))
                    kf = kqpool.tile([P, S], BF16, tag="kf", name="kf")
                    for rh in range(2):
                        ps = ps_mm()
                        for kc in range(C):
                            nc.tensor.matmul(
                                ps[:], wt[:, kc, :],
                                kv_src[:, kc, rh * R:(rh + 1) * R],
                                start=(kc == 0), stop=(kc == C - 1))
                        copy_bias(kf[:, rh * R:(rh + 1) * R], ps[:],
                                  bk_t[:, oc:oc + 1], rh)
                    wtq = wpool.tile([P, C, P], BF16, tag="wstat",
                                     name=f"wstq{oc}")
                    nc.sync.dma_start(out=wtq[:], in_=wstat_ap(w_q, oc, C))
                    qf = kqpool.tile([P, R], BF16, tag="qf", name="qf")
                    ps = ps_mm()
                    for kc in range(C):
                        nc.tensor.matmul(ps[:], wtq[:, kc, :],
                                         q_src[:, kc, 0:R],
                                         start=(kc == 0), stop=(kc == C - 1))
                    copy_bias(qf[:], ps[:], bq_t[:, oc:oc + 1], oc)
                    kq[oc] = (kf, qf)

                def att(oc):
                    kf, qf = kq.pop(oc)
                    ctx_ps = [ps_ctx(f"ctxps{hh}") for hh in range(2)]
                    for kc in range(C):
                        # row-tiled concurrent score pair (heads 2oc, 2oc+1)
                        # into the two banks of one [P, 2R] tile, then one
                        # batched exp for both heads
                        ps_s = ps_score2("scoreAB")
                        for hh in range(2):
                            po = hh * 64
                            nc.tensor.matmul(
                                ps_s[:, hh * R:(hh + 1) * R],
                                kf[po:po + 64, kc * P:(kc + 1) * P],
                                qf[po:po + 64, :], start=True, stop=True)
                        ex = expool.tile([P, 2 * R], BF16, tag="exp",
                                         name="exAB")
                        nc.scalar.activation(ex[:], ps_s[:], AF.Exp,
                                             scale=0.125)
                        for hh in range(2):
                            h = oc * 2 + hh
                            nc.tensor.matmul(
                                ctx_ps[hh][:65, :],
                                v4[:, kc, h, 0:65],
                                ex[:, hh * R:(hh + 1) * R],
                                start=(kc == 0), stop=(kc == C - 1))
                    for hh in range(2):
                        po = hh * 64
                        # custom-DVE ops misread PSUM at partition offsets;
                        # stage the denominator row through SBUF first
                        den = small.tile([1, R], F32, tag="attden",
                                         name="attden")
                        nc.scalar.activation(den[:], ctx_ps[hh][64:65, :],
                                             AF.Copy)
                        rec = small.tile([1, R], F32R, tag="rec", name="rec")
                        recip_r(rec[:], den[:])
                        bc_ps = ps_score("bcast")
                        nc.tensor.matmul(bc_ps[0:64, :],
                                         ones_row_r[0:1, 0:64], rec[:],
                                         start=True, stop=True)
                        bc_sb = small.tile([64, R], BF16, tag="bcsb",
                                           name="bcsb")
                        nc.scalar.activation(bc_sb[:], bc_ps[0:64, :],
                                             AF.Copy)
                        nc.vector.tensor_mul(ctx_t[po:po + 64, oc, :],
                                             ctx_ps[hh][0:64, :], bc_sb[:])

                kq_proj(0)
                kq_proj(1)
                for oc in range(C):
                    if oc + 2 < C:
                        kq_proj(oc + 2)
                    att(oc)
                return ctx_t

            # ---------------- phase 1: SGIRA self-attention ----------------
            ssp_cm = tc.tile_pool(name="ssp", bufs=1, side="right")
            ssp = ssp_cm.__enter__()
            ss = ssp.tile([P, C, R], BF16, tag="ss")
            with tc.tile_pool(name="p1", bufs=1) as p1, \
                 tc.tile_pool(name="p1kq", bufs=2) as p1kq, \
                 tc.tile_pool(name="p1ex", bufs=3) as p1ex, \
                 tc.tile_pool(name="p1wm", bufs=2) as p1wm:
                xT_s = p1.tile([P, C, S], BF16, tag="xT")
                xt_ap = xT.ap().rearrange("(c p) r -> p c r", p=P)
                for c in range(C):
                    nc.sync.dma_start(out=xT_s[:, c, :], in_=xt_ap[:, c, :])
                bq_s = load_pc(bq); bk_s = load_pc(bk)
                bv_s = load_pc(bv) if with_vbias else None
                ctx_sa = mha16(xT_s, xT_s, wq, wk, wv, bq_s, bk_s, bv_s,
                               (p1, p1, p1kq, p1ex, p1wm))
                # out-proj + residual + LN -> ss
                bo_s = load_pc(bo)
                nsg_s = load_pc(nsg); nsb_s = load_pc(nsb)
                tap("ctxsa", ctx_sa)
                sa = p1.tile([P, C, R], BF16, tag="sa")
                proj(sa, C, ctx_sa, C, wo, bo_s)
                for c in range(C):
                    nc.vector.tensor_add(sa[:, c, :], sa[:, c, :],
                                         xT_s[:, c, 0:R])
                layer_norm(sa, C, nsg_s, nsb_s, ss)
                tap("ss", ss)

            # ---------------- optional cross-attention (gate != 1) ---------
            if include_cross:
                fusedp_cm = tc.tile_pool(name="fusedp", bufs=1, side="right")
                fusedp = fusedp_cm.__enter__()
                fused = fusedp.tile([P, C, R], BF16, tag="fused")
                with tc.tile_pool(name="pc1", bufs=1) as pc1, \
                     tc.tile_pool(name="pc1kq", bufs=2) as pc1kq, \
                     tc.tile_pool(name="pc1ex", bufs=3) as pc1ex, \
                     tc.tile_pool(name="pc1wm", bufs=2) as pc1wm:
                    mT_s = pc1.tile([P, C, S], BF16, tag="mT")
                    mt_ap = mT.ap().rearrange("(c p) r -> p c r", p=P)
                    for c in range(C):
                        nc.sync.dma_start(out=mT_s[:, c, :], in_=mt_ap[:, c, :])
                    cbq_s = load_pc(cbq); cbk_s = load_pc(cbk)
                    cbv_s = load_pc(cbv) if with_vbias else None
                    ctx_ca = mha16(mT_s, ss, cwq, cwk, cwv, cbq_s, cbk_s,
                                   cbv_s, (pc1, pc1, pc1kq, pc1ex, pc1wm))
                    cbo_s = load_pc(cbo)
                    ncg_s = load_pc(ncg); ncb_s = load_pc(ncb)
                    gc_s = load_pc(gate_c); g1_s = load_pc(gate_1mc)
                    ca = pc1.tile([P, C, R], BF16, tag="ca")
                    proj(ca, C, ctx_ca, C, cwo, cbo_s)
                    for c in range(C):
                        nc.vector.tensor_add(ca[:, c, :], ca[:, c, :],
                                             ss[:, c, :])
                    cs = pc1.tile([P, C, R], BF16, tag="cs")
                    layer_norm(ca, C, ncg_s, ncb_s, cs)
                    # fused = gate*ss + (1-gate)*cs
                    for c in range(C):
                        nc.vector.tensor_scalar(
                            fused[:, c, :], ss[:, c, :], gc_s[:, 0:1], None,
                            OP.mult)
                        nc.vector.tensor_scalar(
                            cs[:, c, :], cs[:, c, :], g1_s[:, 0:1], None,
                            OP.mult)
                        nc.vector.tensor_add(fused[:, c, :], fused[:, c, :],
                                             cs[:, c, :])
                ff_in = fused
            else:
                ff_in = ss

            # ---------------- phase 2: FFN ----------------
            hidp_cm = tc.tile_pool(name="hidp", bufs=1)
            hidp = hidp_cm.__enter__()
            hidden = hidp.tile([P, C, R], BF16, tag="hidden")
            with tc.tile_pool(name="p2", bufs=1) as p2:
                b1_s = load_pc(b1); b2_s = load_pc(b2)
                nfg_s = load_pc(nfg); nfb_s = load_pc(nfb)
                h1 = p2.tile([P, CFF, R], BF16, tag="h1")
                proj(h1, CFF, ff_in, C, w1, b1_s, func="gelu")
                ffo = p2.tile([P, C, R], BF16, tag="ffo")
                for oc in range(C):
                    wt2 = p2.tile([P, CFF, P], BF16, tag="wstat2", bufs=3,
                                  name=f"wst2_{oc}")
                    nc.sync.dma_start(out=wt2[:], in_=wstat_ap(w2, oc, CFF))
                    ps = ps_mm()
                    for kc in range(CFF):
                        nc.tensor.matmul(ps[:], wt2[:, kc, :], h1[:, kc, :],
                                         start=(kc == 0), stop=(kc == CFF - 1))
                    copy_bias(ffo[:, oc, :], ps[:], b2_s[:, oc:oc + 1], oc)
                for c in range(C):
                    nc.vector.tensor_add(ffo[:, c, :], ffo[:, c, :],
                                         ff_in[:, c, :])
                layer_norm(ffo, C, nfg_s, nfb_s, hidden)
                tap("hidden", hidden)
            # ss (or fused) no longer needed
            if include_cross:
                fusedp_cm.__exit__(None, None, None)
            ssp_cm.__exit__(None, None, None)

            # ---------------- phase 3: SAIGA squeeze-excite ----------------
            sep_cm = tc.tile_pool(name="sep", bufs=1, side="right")
            sep = sep_cm.__enter__()
            se_own = sep.tile([P, C, R], BF16, tag="se_own")
            with tc.tile_pool(name="p3", bufs=1) as p3:
                exb_s = load_pc(exb); sqb_s = load_pc(sqb)
                nrg_s = load_pc(nrg); nrb_s = load_pc(nrb)
                h2 = p3.tile([P, C2, R], BF16, tag="h2")
                proj(h2, C2, hidden, C, exw, exb_s, func="relu")
                sqo = p3.tile([P, C, R], BF16, tag="sqo")
                proj(sqo, C, h2, C2, sqw, sqb_s)
                for c in range(C):
                    nc.vector.tensor_add(sqo[:, c, :], sqo[:, c, :],
                                         hidden[:, c, :])
                layer_norm(sqo, C, nrg_s, nrb_s, se_own)
                tap("se", se_own)
            hidp_cm.__exit__(None, None, None)

            # ------- phase 4: SAIGA K/V own-half + pair AllReduce ----------
            # ------- phase 5: 4-head attention (own half overlaps the cc) --
            with tc.tile_pool(name="p5", bufs=1) as p5, \
                 tc.tile_pool(name="p5ex", bufs=3) as p5ex, \
                 tc.tile_pool(name="p5wm", bufs=2) as p5wm, \
                 tc.tile_pool(name="dramp", bufs=1, space="DRAM") as dramp:
                qb_s = load_pc(qb); kb_s = load_pc(kb)
                vb_s = load_pc(vb) if with_vbias else None

                # K2 feature-major [P, C, S]; own rows first
                k2 = p5.tile([P, C, S], BF16, tag="k2")
                for oc in range(C):
                    wt = wpool.tile([P, C, P], BF16, tag="wstat",
                                    name=f"wstk2_{oc}")
                    nc.sync.dma_start(out=wt[:], in_=wstat_ap(kw, oc, C))
                    ps = ps_mm()
                    for kc in range(C):
                        nc.tensor.matmul(ps[:], wt[:, kc, :],
                                         se_own[:, kc, :],
                                         start=(kc == 0), stop=(kc == C - 1))
                    copy_bias(k2[:, oc, 0:R], ps[:], kb_s[:, oc:oc + 1], oc)
                in_b = dramp.tile([2, P, 8 * R], BF16, tag="cc_in")
                red = dramp.tile([2, P, 8 * R], BF16, tag="cc_out")
                nc.gpsimd.dma_start(
                    out=in_b[0].rearrange("p (c r) -> p c r", c=C),
                    in_=k2[:, :, 0:R])

                # V2 row-major [P, rc, D]; own row chunks 0-3
                v2 = p5.tile([P, C, D], BF16, tag="v2")
                for qh in range(2):
                    wm = p5wm.tile([P, C, R], BF16, tag="wmov",
                                   name=f"wmv2_{qh}")
                    nc.sync.dma_start(
                        out=wm[:],
                        in_=vw.ap().rearrange("(k p) m -> p k m", p=P)
                        [:, :, qh * R:(qh + 1) * R])
                    for rc in range(4):
                        ps = ps_mm()
                        for kc in range(C):
                            nc.tensor.matmul(
                                ps[:], se_own[:, kc, rc * P:(rc + 1) * P],
                                wm[:, kc, :], start=(kc == 0),
                                stop=(kc == C - 1 and vb_s is None))
                        if vb_s is not None:
                            nc.tensor.matmul(
                                ps[:], ones_row[:],
                                vb_s[0:1, qh * R:(qh + 1) * R],
                                start=False, stop=True)
                        if rc % 2 == 0:
                            nc.vector.tensor_scalar(
                                v2[:, rc, qh * R:(qh + 1) * R], ps[:],
                                0.0, None, OP.add)
                        else:
                            nc.scalar.activation(
                                v2[:, rc, qh * R:(qh + 1) * R], ps[:],
                                AF.Copy)
                    nc.gpsimd.dma_start(
                        out=in_b[1].rearrange("p (c r) -> p c r", c=4)
                        [:, :, qh * R:(qh + 1) * R],
                        in_=v2[:, 0:4, qh * R:(qh + 1) * R])
                nc.gpsimd.collective_compute(
                    "AllReduce", OP.add,
                    replica_groups=[[0, 1], [2, 3], [4, 5], [6, 7]],
                    ins=[in_b.opt()], outs=[red.opt()])

                # Q2 projections (independent of the collective)
                q2 = p5.tile([P, C, R], BF16, tag="q2")
                proj(q2, C, se_own, C, qw, qb_s)

                # own-half attention: heads accumulate ctx/denominator over
                # k-chunks 0-3 into SBUF, freeing PSUM before the peer half
                ctx_own = p5.tile([P, C, R], BF16, tag="ctx_own")
                sum_own = p5.tile([1, H_SG, R], F32, tag="sum_own")
                ex_t = {}

                def att2_half(h, rng, ctx_sb, sum_sb, prev_ctx, prev_sum):
                    ctx_ps = [ps_ctx(f"c2ps{mh}") for mh in range(2)]
                    sum_ps = ps_mm("a2sum")
                    for j, kc in enumerate(rng):
                        first, last = (j == 0), (j == len(rng) - 1)
                        ps_s = ps_score("score2")
                        for i in range(2):
                            oc = 2 * h + i
                            nc.tensor.matmul(
                                ps_s[:], k2[:, oc, kc * P:(kc + 1) * P],
                                q2[:, oc, :], start=(i == 0), stop=(i == 1))
                        ex = p5ex.tile([P, R], BF16, tag="exp", name="ex2")
                        nc.scalar.activation(ex[:], ps_s[:], AF.Exp,
                                             scale=0.0625)
                        nc.tensor.matmul(sum_ps[0:1, :], ones_col[:], ex[:],
                                         start=first, stop=last)
                        for mh in range(2):
                            nc.tensor.matmul(
                                ctx_ps[mh][:],
                                v2[:, kc, (h * 256 + mh * P):(h * 256 + (mh + 1) * P)],
                                ex[:], start=first, stop=last)
                    if prev_ctx is None:
                        nc.vector.tensor_scalar(
                            ctx_sb[:, 2 * h, :], ctx_ps[0][:],
                            0.0, None, OP.add)
                        nc.scalar.activation(ctx_sb[:, 2 * h + 1, :],
                                             ctx_ps[1][:], AF.Copy)
                        nc.vector.tensor_scalar(sum_sb[0:1, h, :],
                                                sum_ps[0:1, :], 0.0, None,
                                                OP.add)
                    else:
                        # combine halves, normalize, write ctx
                        den = small.tile([1, R], F32, tag="a2den", name="den")
                        nc.vector.tensor_add(den[:], sum_ps[0:1, :],
                                             prev_sum[0:1, h, :])
                        rec = small.tile([1, R], F32R, tag="rec", name="rec2")
                        recip_r(rec[:], den[:])
                        bc_ps = bcast_rows(rec)
                        bc_sb = small.tile([P, R], BF16, tag="bcsb",
                                           name="bcsb2")
                        nc.scalar.activation(bc_sb[:], bc_ps[:], AF.Copy)
                        for mh in range(2):
                            oc = 2 * h + mh
                            tot = small.tile([P, R], BF16, tag="a2tot",
                                             name="tot")
                            nc.vector.tensor_add(tot[:], ctx_ps[mh][:],
                                                 prev_ctx[:, oc, :])
                            nc.vector.tensor_mul(ctx_sb[:, oc, :], tot[:],
                                                 bc_sb[:])

                for h in range(H_SG):
                    att2_half(h, range(4), ctx_own, sum_own, None, None)

                # peer recovery: peer = allreduce_sum - own
                ksum = p5.tile([P, C, R], BF16, tag="ksum")
                for half in range(2):
                    nc.sync.dma_start(
                        out=ksum[:, half * 4:(half + 1) * 4, :],
                        in_=red[0].rearrange("p (c r) -> p c r", c=C)
                        [:, half * 4:(half + 1) * 4, :])
                    for oc in range(half * 4, (half + 1) * 4):
                        nc.vector.tensor_sub(k2[:, oc, R:S], ksum[:, oc, :],
                                             k2[:, oc, 0:R])
                vsum = p5.tile([P, 4, D], BF16, tag="vsum")
                nc.sync.dma_start(
                    out=vsum[:],
                    in_=red[1].rearrange("p (c r) -> p c r", c=4))
                for rc in range(4):
                    nc.vector.tensor_sub(v2[:, 4 + rc, :], vsum[:, rc, :],
                                         v2[:, rc, :])

                ctx2 = p5.tile([P, C, R], BF16, tag="ctx2")
                for h in range(H_SG):
                    att2_half(h, range(4, 8), ctx2, None, ctx_own, sum_own)

                tap("ctx2", ctx2)
                # ---------------- phase 6: final residual + LN -------------
                for c in range(C):
                    nc.vector.tensor_add(ctx2[:, c, :], ctx2[:, c, :],
                                         se_own[:, c, :])
                fin = p5.tile([P, C, R], F32, tag="fin")
                layer_norm(ctx2, C, nrg_s, nrb_s, fin)
                out_ap = out_d.ap().rearrange("(c p) r -> p c r", p=P)
                for c in range(C):
                    nc.sync.dma_start(out=out_ap[:, c, :], in_=fin[:, c, :])
            sep_cm.__exit__(None, None, None)

    nc.compile()
    return nc


def _pc(v):
    """[n*128] -> [128, n] per-partition layout."""
    v = np.asarray(v, np.float32)
    return np.ascontiguousarray(v.reshape(-1, P).T)


def _bf(a):
    return np.ascontiguousarray(np.asarray(a, np.float32)
                                .astype(ml_dtypes.bfloat16))


def _f8(a):
    """fp8e4m3 weights pre-scaled by WSC (clipped to the TRN ±240 range)."""
    v = np.asarray(a, np.float32) * WSC
    return np.ascontiguousarray(np.clip(v, -240.0, 240.0)
                                .astype(ml_dtypes.float8_e4m3))


def kernel(**inputs):
    x = np.asarray(inputs["input_states"], np.float32)
    gate = float(np.asarray(inputs["gate"]).ravel()[0])
    include_cross = (gate != 1.0)

    bq, bk, bv = np.split(np.asarray(inputs["sa_in_b"], np.float32), 3)
    vb = np.asarray(inputs["v_b"], np.float32)
    cbv = (np.split(np.asarray(inputs["ca_in_b"], np.float32), 3)[2]
           if include_cross else np.zeros(1, np.float32))
    with_vbias = bool(np.any(bv) or np.any(vb) or np.any(cbv))

    key = (include_cross, with_vbias)
    if key not in _CACHE:
        _CACHE[key] = _build(include_cross, with_vbias)
    nc = _CACHE[key]

    wq, wk, wv = [_bf(w) for w in
                  np.split(np.asarray(inputs["sa_in_w"], np.float32), 3,
                           axis=1)]

    shared = {
        "wq": wq, "wk": wk, "wv": wv,
        "bq": _pc(bq), "bk": _pc(bk),
        "wo": _bf(inputs["sa_out_w"]),
        "bo": _pc(inputs["sa_out_b"]),
        "w1": _bf(inputs["ffn_w1"]),
        "b1": _pc(inputs["ffn_b1"]),
        "w2": _bf(inputs["ffn_w2"]),
        "b2": _pc(inputs["ffn_b2"]),
        "exw": _bf(inputs["ex_w"]),
        "exb": _pc(inputs["ex_b"]),
        "sqw": _bf(inputs["sq_w"]),
        "sqb": _pc(inputs["sq_b"]),
        "qw": _bf(inputs["q_w"]),
        "qb": _pc(inputs["q_b"]),
        "kw": _bf(inputs["k_w"]),
        "kb": _pc(inputs["k_b"]),
        "vw": _bf(inputs["v_w"]),
        "nsg": _pc(inputs["ns_g"]), "nsb": _pc(inputs["ns_b"]),
        "nfg": _pc(inputs["nf_g"]), "nfb": _pc(inputs["nf_b"]),
        "nrg": _pc(inputs["nrm_g"]), "nrb": _pc(inputs["nrm_b"]),
    }
    if with_vbias:
        shared["bv"] = _bf(bv.reshape(1, D))
        shared["vb"] = _bf(vb.reshape(1, D))
    if include_cross:
        m = np.asarray(inputs["memory_states"], np.float32)
        cwq, cwk, cwv = [_bf(w) for w in
                         np.split(np.asarray(inputs["ca_in_w"], np.float32),
                                  3, axis=1)]
        cbq, cbk, cbv_ = np.split(np.asarray(inputs["ca_in_b"], np.float32), 3)
        shared.update({
            "cwq": cwq, "cwk": cwk, "cwv": cwv,
            "cbq": _pc(cbq), "cbk": _pc(cbk),
            "cwo": _bf(inputs["ca_out_w"]),
            "cbo": _pc(inputs["ca_out_b"]),
            "ncg": _pc(inputs["nc_g"]), "ncb": _pc(inputs["nc_b"]),
            "gate_c": np.full((P, 1), gate, np.float32),
            "gate_1mc": np.full((P, 1), 1.0 - gate, np.float32),
        })
        if with_vbias:
            shared["cbv"] = _bf(cbv_.reshape(1, D))

    in_maps = []
    for c in range(N_CORES):
        b, hf = c // 2, c % 2
        xp = np.concatenate([x[b, hf * R:(hf + 1) * R],
                             x[b, (1 - hf) * R:(2 - hf) * R]], axis=0)
        m_in = dict(shared)
        m_in["xT"] = _bf(xp.T)
        if include_cross:
            m_in["mT"] = _bf(m[b].T)
        in_maps.append(m_in)

    res = bass_utils.run_bass_kernel_spmd(nc, in_maps,
                                          core_ids=list(range(N_CORES)))
    out = np.empty((4, S, D), np.float32)
    for c in range(N_CORES):
        b, hf = c // 2, c % 2
        out[b, hf * R:(hf + 1) * R, :] = res.results[c]["out"].T
    return out
